# revision 1
# baseline (speedup 1.0000x reference)
"""Fused sparse-attention kernel for Trainium2 (8 NeuronCores, data-parallel over batch).

Computation (per batch element b):
    X[s,k]  = enc[b] @ W_enc + dec_proj[b,k] + cov[b,s]*Wcovsum[k] + bias[k]
    T       = tanh(X)
    att[s]  = T @ v_w                      (+ v_b, which cancels in softmax)
    w       = softmax(att masked to s < len[b])
    new_cov = cov + w

Sharding: batch B=32 is split 4-per-core across 8 cores; all weights replicated
(per the data-parallel sharding hint).

Pipeline per batch element (per core, 4 batch elements):
  1. SWDGE cast-DMA: enc[b] fp32 DRAM -> bf16 DRAM scratch (64KB descriptors;
     the DMA compute path does the fp32->bf16 rounding for free).
  2. xbar DMA-transpose: bf16 DRAM -> SBUF enc^T tiles [h,s] (h on partitions).
     (The matmul contraction dim must live on partitions for both operands and
     the xbar only handles 2-byte dtypes from a contiguous source, hence the
     bf16 bounce; SBUF-sourced transposes hang the device, and strided/
     partition-stepped matmul APs are rejected by the BIR verifier.)
  3. PE: per s-tile psum group = K=2 rank-1 (ones, cov) x (bias_b, Wcovsum)
     + 4x K=128 bf16 matmuls against W_enc chunks (X[s,k] orientation so the
     v-reduction lands on the DVE, keeping PE to ~69us/core).
  4. ACT: tanh psum -> bf16 T tiles.
  5. DVE: fused T*v multiply + free-dim reduce (scalar_tensor_tensor accum_out)
     -> att column [128,1].
  6. Tiny masked softmax tail in [s_lo=128, s_hi=16] layout: exp on ACT,
     iota<len mask fused with the exp multiply on DVE, sum + 1/sum broadcast
     via two small PE matmuls (softmax max-subtraction is skipped: |logits| <=
     ||v||_1 ~ 8, safely inside fp32 exp range, and v_b cancels in softmax).
"""

import numpy as np
import ml_dtypes

B, S, H, E = 32, 2048, 512, 512
NCORES = 8
BPC = B // NCORES           # batches per core
SLO, SHI = 128, S // 128    # att tile layout: s = 128*j + p  ->  [p, j]
HC = H // 128               # h chunks
WB_C = H + H + BPC          # per-chunk columns in the bf16 weight blob
BF16 = ml_dtypes.bfloat16

_CACHE = {}


def _build_nc():
    import concourse.mybir as mybir
    import concourse.tile as tile
    from concourse import bacc
    from contextlib import ExitStack

    dt = mybir.dt
    F32, BF = dt.float32, dt.bfloat16

    nc = bacc.Bacc("TRN2", target_bir_lowering=False, debug=False,
                   enable_asserts=False, num_devices=NCORES)

    # ---- DRAM I/O (per-core shapes) ----
    enc_f32 = nc.dram_tensor("enc_f32", [BPC, S, H], F32, kind="ExternalInput").ap()
    # bf16 blob: per chunk c: [wenc_c (H) | ws_c (H) | decT_c (BPC)] then ones col
    wblob = nc.dram_tensor("wblob", [128, HC * WB_C + 1], BF, kind="ExternalInput").ap()
    # f32 blob: [iota (SHI) | lens (BPC) | cov_t (BPC*SHI)]
    fblob = nc.dram_tensor("fblob", [SLO, SHI + BPC + BPC * SHI], F32,
                           kind="ExternalInput").ap()
    r1lhs = nc.dram_tensor("r1lhs", [2, BPC * S], BF, kind="ExternalInput").ap()
    vbc = nc.dram_tensor("vbc", [128, H], BF, kind="ExternalInput").ap()
    wcov4 = nc.dram_tensor("wcov4", [1, BPC * H], BF, kind="ExternalInput").ap()
    # row consts: [b (H) | ones_r (128)]
    brow = nc.dram_tensor("brow", [1, H + 128], F32, kind="ExternalInput").ap()
    att_out = nc.dram_tensor("att_out", [BPC, SLO, SHI], F32, kind="ExternalOutput").ap()
    cov_out = nc.dram_tensor("cov_out", [BPC, SLO, SHI], F32, kind="ExternalOutput").ap()

    AF = mybir.ActivationFunctionType
    OP = mybir.AluOpType

    with tile.TileContext(nc) as tc, ExitStack() as ctx:
        consts = ctx.enter_context(tc.tile_pool(name="consts", bufs=1))
        encp = ctx.enter_context(tc.tile_pool(name="encp", bufs=3))
        tpool = ctx.enter_context(tc.tile_pool(name="tpool", bufs=4))
        spool = ctx.enter_context(tc.tile_pool(name="spool", bufs=2))
        small = ctx.enter_context(tc.tile_pool(name="small", bufs=2))
        attp = ctx.enter_context(tc.tile_pool(name="attp", bufs=2))
        ppm = ctx.enter_context(tc.tile_pool(name="ppm", bufs=5, space="PSUM"))
        pps = ctx.enter_context(tc.tile_pool(name="pps", bufs=1, space="PSUM"))
        dramp = ctx.enter_context(tc.tile_pool(name="dramp", bufs=2, space="DRAM"))

        # ---- per-batch loads. Half-batch DRAM scratch tiles (bufs=1) both
        # throttle the SWDGE casts (WAR on the slot) and let the first
        # transposes start after only half a batch is cast. ----
        def load_batch(b):
            sh = S // 2
            enc_t = []
            for c in range(HC):
                e_t = encp.tile([128, S], BF, tag=f"enc{c}")
                enc_t.append(e_t)
            for j in range(2):
                enc16 = dramp.tile([sh, H], BF, tag=f"enc16{j}")
                nc.gpsimd.dma_start(
                    enc16[:].rearrange("a b -> (a b)"),
                    enc_f32[b, j * sh:(j + 1) * sh].rearrange("a b -> (a b)"))
                for c in range(HC):
                    nc.sync.dma_start(
                        enc_t[c][:, j * sh:(j + 1) * sh],
                        enc16[:, c * 128:(c + 1) * 128],
                        transpose=True)
            return enc_t

        # ---- one-time constant loads (6 DMAs). Emitted on the Pool (SWDGE)
        # queue BEFORE the enc casts so their DMA requests precede the flood. ----
        wb_sb = consts.tile([128, HC * WB_C + 1], BF, tag="wblob")
        nc.gpsimd.dma_start(wb_sb[:], wblob[:])
        fb_sb = consts.tile([SLO, SHI + BPC + BPC * SHI], F32, tag="fblob")
        nc.gpsimd.dma_start(fb_sb[:], fblob[:])
        r1lhs_sb = consts.tile([2, BPC * S], BF, tag="r1lhs")
        nc.gpsimd.dma_start(r1lhs_sb[:], r1lhs[:])
        vbc_sb = consts.tile([128, H], BF, tag="vbc")
        nc.gpsimd.dma_start(vbc_sb[:], vbc[:])
        brow_sb = consts.tile([1, H + 128], F32, tag="brow")
        nc.gpsimd.dma_start(brow_sb[:], brow[:])
        r1rhs_sb = consts.tile([2, BPC * H], BF, tag="r1rhs")
        nc.gpsimd.dma_start(r1rhs_sb[1:2, :], wcov4[:])

        pre = {0: load_batch(0)}

        def wenc_sb(c):
            return wb_sb[:, c * WB_C:c * WB_C + H]

        def ws_sb(c):
            return wb_sb[:, c * WB_C + H:c * WB_C + 2 * H]

        def decT_sb(c, b):
            return wb_sb[:, c * WB_C + 2 * H + b:c * WB_C + 2 * H + b + 1]

        ones_c_sb = wb_sb[:, HC * WB_C:HC * WB_C + 1]
        iota_sb = fb_sb[:, 0:SHI]
        lens_sb = fb_sb[:, SHI:SHI + BPC]
        covt_sb = fb_sb[:, SHI + BPC:]
        b_row_sb = brow_sb[:, 0:H]
        ones_r_sb = brow_sb[:, H:H + 128]

        # ---- rank-1 rhs row0 per batch: dec_proj[b] + b ----
        for b in range(BPC):
            dp_ps = pps.tile([1, H], F32, tag="dp")
            for c in range(HC):
                nc.tensor.matmul(dp_ps[:], decT_sb(c, b), ws_sb(c),
                                 start=(c == 0), stop=(c == HC - 1))
            nc.vector.tensor_tensor(r1rhs_sb[0:1, b * H:(b + 1) * H],
                                    dp_ps[:], b_row_sb, OP.add)

        # ---- main loop ----
        for b in range(BPC):
            enc_t = pre.pop(b)
            if b + 1 < BPC:
                pre[b + 1] = load_batch(b + 1)

            att_t = attp.tile([SLO, SHI], F32, tag="att")
            for j in range(SHI):
                ps = ppm.tile([128, H], F32, tag="x")
                nc.tensor.matmul(
                    ps[:],
                    r1lhs_sb[:, b * S + j * 128: b * S + (j + 1) * 128],
                    r1rhs_sb[:, b * H:(b + 1) * H],
                    start=True, stop=False,
                )
                for c in range(HC):
                    nc.tensor.matmul(
                        ps[:],
                        enc_t[c][:, j * 128:(j + 1) * 128],
                        wenc_sb(c),
                        start=False, stop=(c == HC - 1),
                    )
                t_t = tpool.tile([128, H], BF, tag="t")
                nc.scalar.activation(t_t[:], ps[:], AF.Tanh)
                scr = spool.tile([128, H], BF, tag="scr")
                nc.vector.scalar_tensor_tensor(
                    out=scr[:], in0=t_t[:], scalar=1.0, in1=vbc_sb[:],
                    op0=OP.mult, op1=OP.mult,
                    accum_out=att_t[:, j:j + 1],
                )

            # ---- masked softmax tail (tiny) ----
            expt = small.tile([SLO, SHI], F32, tag="expt")
            nc.scalar.activation(expt[:], att_t[:], AF.Exp)
            mexp = small.tile([SLO, SHI], F32, tag="mexp")
            nc.vector.scalar_tensor_tensor(
                out=mexp[:], in0=iota_sb, scalar=lens_sb[:, b:b + 1],
                in1=expt[:], op0=OP.is_lt, op1=OP.mult,
            )
            mexp16 = small.tile([SLO, SHI], BF, tag="mexp16")
            nc.vector.tensor_copy(mexp16[:], mexp[:])
            sum_ps = pps.tile([1, SHI], F32, tag="sum")
            nc.tensor.matmul(sum_ps[:], ones_c_sb, mexp16[:],
                             start=True, stop=True)
            ssum = small.tile([1, 1], F32, tag="ssum")
            nc.vector.reduce_sum(ssum[:], sum_ps[:], axis=mybir.AxisListType.X)
            sinv = small.tile([1, 1], F32, tag="sinv")
            nc.vector.reciprocal(sinv[:], ssum[:])
            inv_ps = pps.tile([128, 1], F32, tag="inv")
            nc.tensor.matmul(inv_ps[:], ones_r_sb, sinv[:], start=True, stop=True)
            wts = small.tile([SLO, SHI], F32, tag="wts")
            nc.vector.tensor_scalar(wts[:], mexp[:], inv_ps[:], None, OP.mult)
            nc.scalar.dma_start(att_out[b], wts[:])
            ncov = small.tile([SLO, SHI], F32, tag="ncov")
            nc.vector.tensor_tensor(ncov[:], wts[:],
                                    covt_sb[:, b * SHI:(b + 1) * SHI], OP.add)
            nc.scalar.dma_start(cov_out[b], ncov[:])

    nc.compile()
    return nc


def _get_nc():
    if "nc" not in _CACHE:
        _CACHE["nc"] = _build_nc()
    return _CACHE["nc"]


def _prep_in_maps(dec_input, enc_output, text_lengths, coverage_vector, W, b, v_w):
    enc = np.ascontiguousarray(np.asarray(enc_output, dtype=np.float32))
    dec = np.asarray(dec_input, dtype=np.float32).reshape(B, E)
    cov = np.asarray(coverage_vector, dtype=np.float32)
    W = np.asarray(W, dtype=np.float32)
    b = np.asarray(b, dtype=np.float32)
    v_w = np.asarray(v_w, dtype=np.float32)
    lens_f = np.asarray(text_lengths).astype(np.float32)

    wenc = W[:H].astype(BF16)                   # [h, k]
    ws = W[H:H + E].astype(BF16)                # [e, k]
    wcov = W[H + E:].sum(axis=0, dtype=np.float32).astype(BF16)  # [k]
    decT = dec.T.astype(BF16)                   # [e, B]
    vbc = np.ascontiguousarray(np.broadcast_to(v_w.astype(BF16), (128, H)))
    iota = (np.arange(SLO, dtype=np.float32)[:, None]
            + 128.0 * np.arange(SHI, dtype=np.float32)[None, :])

    brow = np.empty((1, H + 128), np.float32)
    brow[0, :H] = b
    brow[0, H:] = 1.0

    wcov4 = np.ascontiguousarray(np.broadcast_to(
        wcov[None, :], (BPC, H)).reshape(1, BPC * H))

    in_maps = []
    for core in range(NCORES):
        sl = slice(core * BPC, (core + 1) * BPC)
        wblob = np.zeros((128, HC * WB_C + 1), BF16)
        for c in range(HC):
            o = c * WB_C
            wblob[:, o:o + H] = wenc[c * 128:(c + 1) * 128]
            wblob[:, o + H:o + 2 * H] = ws[c * 128:(c + 1) * 128]
            wblob[:, o + 2 * H:o + 2 * H + BPC] = decT[c * 128:(c + 1) * 128, sl]
        wblob[:, HC * WB_C] = BF16(1.0)

        fblob = np.empty((SLO, SHI + BPC + BPC * SHI), np.float32)
        fblob[:, 0:SHI] = iota
        fblob[:, SHI:SHI + BPC] = lens_f[sl][None, :]
        fblob[:, SHI + BPC:] = (cov[sl].reshape(BPC, SHI, SLO)
                                .transpose(2, 0, 1).reshape(SLO, BPC * SHI))

        r1 = np.empty((2, BPC * S), BF16)
        r1[0] = BF16(1.0)
        r1[1] = cov[sl].astype(BF16).reshape(-1)

        in_maps.append({
            "enc_f32": enc[sl],
            "wblob": wblob,
            "fblob": fblob,
            "r1lhs": r1,
            "vbc": vbc,
            "wcov4": wcov4,
            "brow": brow,
        })
    return in_maps


def kernel(dec_input, enc_output, text_lengths, coverage_vector, W, b, v_w, v_b):
    from concourse.bass_utils import run_bass_kernel_spmd

    nc = _get_nc()
    in_maps = _prep_in_maps(dec_input, enc_output, text_lengths,
                            coverage_vector, W, b, v_w)
    res = run_bass_kernel_spmd(nc, in_maps, core_ids=list(range(NCORES)))

    att = np.empty((B, S), np.float32)
    ncov = np.empty((B, S), np.float32)
    for core in range(NCORES):
        r = res.results[core]
        att[core * BPC:(core + 1) * BPC] = \
            r["att_out"].transpose(0, 2, 1).reshape(BPC, S)
        ncov[core * BPC:(core + 1) * BPC] = \
            r["cov_out"].transpose(0, 2, 1).reshape(BPC, S)
    return att, ncov



# revision 10
# speedup vs baseline: 1.2340x; 1.2340x over previous
"""Fused sparse-attention kernel for Trainium2 (8 NeuronCores, data-parallel over batch).

Computation (per batch element b):
    X[s,k]  = enc[b] @ W_enc + dec_proj[b,k] + cov[b,s]*Wcovsum[k] + bias[k]
    T       = tanh(X)
    att[s]  = T @ v_w                      (+ v_b, which cancels in softmax)
    w       = softmax(att masked to s < len[b])
    new_cov = cov + w
Sharding: batch B=32 split 4-per-core across 8 cores; weights replicated.

Layout strategy: enc is cast to bf16 and transposed to [128p, HC, S] chunk
layout ON THE HOST, so the device sees one contiguous [128, 16KB] load per
batch element (full DMA rate) instead of the fp32->bf16 DRAM bounce + xbar
DMA-transpose (which tripled HBM traffic and ran at the slow xbar tile rate).
dec_proj (dec @ W_s, 17 MFLOP total) and Wcovsum are folded into the rank-1
rhs on the host; bias b is folded into the same row.

Device pipeline per s-tile (128 positions):
  PE:  psum[s,k] = K=2 rank-1 ([ones;cov] x [dec_proj+b; Wcovsum])
       + enc^T chunks @ W_enc chunks (fp8 e4m3 DoubleRow pairs: K=256 per
       pass at 0.5 cyc/row => 2x PE; operands pre-scaled enc/4, W*16 on the
       host to dodge e4m3 subnormals, rank-1 rhs *4 to match, undone by the
       tanh's scale=0.25)
  ACT: T = tanh(0.25*psum) -> bf16
  DVE: att col = sum_k T*v (scalar_tensor_tensor accum_out)
Masked-softmax tail per batch in [s_lo=128, s_hi=16] layout: exp on ACT,
iota<len mask fused with the exp multiply on DVE, sum + 1/sum broadcast via
two tiny PE matmuls (max-subtraction skipped: |logits| <= ||v||_1 ~ 8).
"""

import numpy as np
import ml_dtypes

B, S, H, E = 32, 2048, 512, 512
NCORES = 8
BPC = B // NCORES           # batches per core
SLO, SHI = 128, S // 128    # att tile layout: s = 128*j + p  ->  [p, j]
HC = H // 128               # h chunks
BF16 = ml_dtypes.bfloat16

USE_FP8 = False
FP8 = ml_dtypes.float8_e4m3fn
ENC_SCALE = 0.25            # enc pre-scale (host)
W_SCALE = 16.0              # W_enc pre-scale (host)
PSUM_SCALE = ENC_SCALE * W_SCALE  # net scale on psum; undone in tanh

_CACHE = {}


def _build_nc():
    import concourse.mybir as mybir
    import concourse.tile as tile
    from concourse import bacc
    from contextlib import ExitStack

    dt = mybir.dt
    F32, BF = dt.float32, dt.bfloat16
    ENC_DT = dt.float8e4 if USE_FP8 else BF

    nc = bacc.Bacc("TRN2", target_bir_lowering=False, debug=False,
                   enable_asserts=False, num_devices=NCORES)

    # ---- DRAM I/O (per-core shapes) ----
    # encT[b, p, c*S + s] = enc[b, s, 128c+p]  (pre-scaled when fp8)
    encT = nc.dram_tensor("encT", [BPC, 128, HC * S], ENC_DT,
                          kind="ExternalInput").ap()
    # wblob: wenc chunk c at cols [c*H, (c+1)*H): wenc[c][p, k] = W[128c+p, k]
    wblob = nc.dram_tensor("wblob", [128, HC * H], ENC_DT,
                           kind="ExternalInput").ap()
    # ones col (bf16) for the softmax-sum matmul
    onesc = nc.dram_tensor("onesc", [128, 1], BF, kind="ExternalInput").ap()
    # f32 blob: [iota (SHI) | lens (BPC) | ones_row (128, row 0) | cov_t (BPC*SHI)]
    fblob = nc.dram_tensor("fblob", [SLO, SHI + BPC + 128 + BPC * SHI], F32,
                           kind="ExternalInput").ap()
    r1lhs = nc.dram_tensor("r1lhs", [2, BPC * S], BF, kind="ExternalInput").ap()
    # r1rhs row0 per batch: (dec_proj[b]+b)*PS ; row1: Wcovsum*PS
    r1rhs = nc.dram_tensor("r1rhs", [2, BPC * H], BF, kind="ExternalInput").ap()
    vbc = nc.dram_tensor("vbc", [128, H], BF, kind="ExternalInput").ap()
    att_out = nc.dram_tensor("att_out", [BPC, SLO, SHI], F32, kind="ExternalOutput").ap()
    cov_out = nc.dram_tensor("cov_out", [BPC, SLO, SHI], F32, kind="ExternalOutput").ap()

    AF = mybir.ActivationFunctionType
    OP = mybir.AluOpType
    DR = mybir.MatmulPerfMode.DoubleRow

    with tile.TileContext(nc) as tc, ExitStack() as ctx:
        consts = ctx.enter_context(tc.tile_pool(name="consts", bufs=1))
        encp = ctx.enter_context(tc.tile_pool(name="encp", bufs=2))
        tpool = ctx.enter_context(tc.tile_pool(name="tpool", bufs=4))
        spool = ctx.enter_context(tc.tile_pool(name="spool", bufs=2))
        small = ctx.enter_context(tc.tile_pool(name="small", bufs=2))
        attp = ctx.enter_context(tc.tile_pool(name="attp", bufs=2))
        ppm = ctx.enter_context(tc.tile_pool(name="ppm", bufs=5, space="PSUM"))
        pps = ctx.enter_context(tc.tile_pool(name="pps", bufs=1, space="PSUM"))

        # ---- one-time constant loads (small; on the Pool queue so their DMA
        # requests precede the enc stream) ----
        wb_sb = consts.tile([128, HC * H], ENC_DT, tag="wblob")
        nc.gpsimd.dma_start(wb_sb[:], wblob[:])
        onesc_sb = consts.tile([128, 1], BF, tag="onesc")
        nc.gpsimd.dma_start(onesc_sb[:], onesc[:])
        fb_sb = consts.tile([SLO, SHI + BPC + 128 + BPC * SHI], F32, tag="fblob")
        nc.gpsimd.dma_start(fb_sb[:], fblob[:])
        r1lhs_sb = consts.tile([2, BPC * S], BF, tag="r1lhs")
        nc.gpsimd.dma_start(r1lhs_sb[:], r1lhs[:])
        r1rhs_sb = consts.tile([2, BPC * H], BF, tag="r1rhs")
        nc.gpsimd.dma_start(r1rhs_sb[:], r1rhs[:])
        vbc_sb = consts.tile([128, H], BF, tag="vbc")
        nc.gpsimd.dma_start(vbc_sb[:], vbc[:])

        iota_sb = fb_sb[:, 0:SHI]
        lens_sb = fb_sb[:, SHI:SHI + BPC]
        covt_sb = fb_sb[:, SHI + BPC + 128:]
        ones_r_sb = fb_sb[0:1, SHI + BPC:SHI + BPC + 128]

        # ---- per-batch enc^T load: 2 contiguous-footprint DMAs (s halves so
        # the first s-tiles can start after half a batch is resident) ----
        def load_batch(b):
            e_t = encp.tile([128, HC, S], ENC_DT, tag="enc")
            sh = S // 2
            src = encT[b].rearrange("p (c s) -> p c s", c=HC)
            for j in range(2):
                nc.sync.dma_start(e_t[:, :, j * sh:(j + 1) * sh],
                                  src[:, :, j * sh:(j + 1) * sh])
            return e_t

        pre = {0: load_batch(0)}

        wb3 = wb_sb[:].rearrange("p (c k) -> p c k", c=HC)

        # ---- main loop ----
        for b in range(BPC):
            enc_t = pre.pop(b)
            if b + 1 < BPC:
                pre[b + 1] = load_batch(b + 1)

            att_t = attp.tile([SLO, SHI], F32, tag="att")
            for j in range(SHI):
                ps = ppm.tile([128, H], F32, tag="x")
                nc.tensor.matmul(
                    ps[:],
                    r1lhs_sb[:, b * S + j * 128: b * S + (j + 1) * 128],
                    r1rhs_sb[:, b * H:(b + 1) * H],
                    start=True, stop=False,
                )
                if USE_FP8:
                    for c in range(0, HC, 2):
                        nc.tensor.matmul(
                            ps[:],
                            enc_t[:, c:c + 2, j * 128:(j + 1) * 128],
                            wb3[:, c:c + 2, :],
                            start=False, stop=(c + 2 == HC),
                            perf_mode=DR,
                        )
                else:
                    for c in range(HC):
                        nc.tensor.matmul(
                            ps[:],
                            enc_t[:, c, j * 128:(j + 1) * 128],
                            wb3[:, c, :],
                            start=False, stop=(c == HC - 1),
                        )
                t_t = tpool.tile([128, H], BF, tag="t")
                nc.scalar.activation(t_t[:], ps[:], AF.Tanh,
                                     scale=1.0 / PSUM_SCALE if USE_FP8 else 1.0)
                scr = spool.tile([128, H], BF, tag="scr")
                nc.vector.scalar_tensor_tensor(
                    out=scr[:], in0=t_t[:], scalar=1.0, in1=vbc_sb[:],
                    op0=OP.mult, op1=OP.mult,
                    accum_out=att_t[:, j:j + 1],
                )

            # ---- masked softmax tail (tiny) ----
            expt = small.tile([SLO, SHI], F32, tag="expt")
            nc.scalar.activation(expt[:], att_t[:], AF.Exp)
            mexp = small.tile([SLO, SHI], F32, tag="mexp")
            nc.vector.scalar_tensor_tensor(
                out=mexp[:], in0=iota_sb, scalar=lens_sb[:, b:b + 1],
                in1=expt[:], op0=OP.is_lt, op1=OP.mult,
            )
            mexp16 = small.tile([SLO, SHI], BF, tag="mexp16")
            nc.vector.tensor_copy(mexp16[:], mexp[:])
            sum_ps = pps.tile([1, SHI], F32, tag="sum")
            nc.tensor.matmul(sum_ps[:], onesc_sb[:], mexp16[:],
                             start=True, stop=True)
            ssum = small.tile([1, 1], F32, tag="ssum")
            nc.vector.reduce_sum(ssum[:], sum_ps[:], axis=mybir.AxisListType.X)
            sinv = small.tile([1, 1], F32, tag="sinv")
            nc.vector.reciprocal(sinv[:], ssum[:])
            inv_ps = pps.tile([128, 1], F32, tag="inv")
            nc.tensor.matmul(inv_ps[:], ones_r_sb, sinv[:], start=True, stop=True)
            wts = small.tile([SLO, SHI], F32, tag="wts")
            nc.vector.tensor_scalar(wts[:], mexp[:], inv_ps[:], None, OP.mult)
            nc.scalar.dma_start(att_out[b], wts[:])
            ncov = small.tile([SLO, SHI], F32, tag="ncov")
            nc.vector.tensor_tensor(ncov[:], wts[:],
                                    covt_sb[:, b * SHI:(b + 1) * SHI], OP.add)
            nc.scalar.dma_start(cov_out[b], ncov[:])

    nc.compile()
    return nc


def _get_nc():
    if "nc" not in _CACHE:
        _CACHE["nc"] = _build_nc()
    return _CACHE["nc"]


def _prep_in_maps(dec_input, enc_output, text_lengths, coverage_vector, W, b, v_w):
    enc = np.asarray(enc_output, dtype=np.float32)
    dec = np.asarray(dec_input, dtype=np.float32).reshape(B, E)
    cov = np.asarray(coverage_vector, dtype=np.float32)
    W = np.asarray(W, dtype=np.float32)
    b = np.asarray(b, dtype=np.float32)
    v_w = np.asarray(v_w, dtype=np.float32)
    lens_f = np.asarray(text_lengths).astype(np.float32)

    enc_dt = FP8 if USE_FP8 else BF16
    ps = PSUM_SCALE if USE_FP8 else 1.0

    # enc^T chunk layout [B, 128, HC, S], host-cast (+pre-scale for fp8)
    if USE_FP8:
        encT = (enc * ENC_SCALE).reshape(B, S, HC, 128).transpose(0, 3, 2, 1)
    else:
        encT = enc.reshape(B, S, HC, 128).transpose(0, 3, 2, 1)
    encT = np.ascontiguousarray(encT).astype(enc_dt).reshape(B, 128, HC * S)

    wenc = W[:H] * (W_SCALE if USE_FP8 else 1.0)      # (H, H)
    wblob = np.ascontiguousarray(
        wenc.reshape(HC, 128, H).transpose(1, 0, 2).reshape(128, HC * H)
    ).astype(enc_dt)

    dec_proj = dec @ W[H:H + E] + b                   # (B, H)
    wcovsum = W[H + E:].sum(axis=0, dtype=np.float32)  # (H,)

    vbc = np.ascontiguousarray(np.broadcast_to(v_w.astype(BF16), (128, H)))
    onesc = np.ones((128, 1), BF16)
    iota = (np.arange(SLO, dtype=np.float32)[:, None]
            + 128.0 * np.arange(SHI, dtype=np.float32)[None, :])

    in_maps = []
    for core in range(NCORES):
        sl = slice(core * BPC, (core + 1) * BPC)

        fblob = np.empty((SLO, SHI + BPC + 128 + BPC * SHI), np.float32)
        fblob[:, 0:SHI] = iota
        fblob[:, SHI:SHI + BPC] = lens_f[sl][None, :]
        fblob[:, SHI + BPC:SHI + BPC + 128] = 1.0
        fblob[:, SHI + BPC + 128:] = (cov[sl].reshape(BPC, SHI, SLO)
                                      .transpose(2, 0, 1).reshape(SLO, BPC * SHI))

        r1l = np.empty((2, BPC * S), BF16)
        r1l[0] = BF16(1.0)
        r1l[1] = cov[sl].astype(BF16).reshape(-1)

        r1r = np.empty((2, BPC * H), np.float32)
        r1r[0] = (dec_proj[sl] * ps).reshape(-1)
        r1r[1] = np.broadcast_to(wcovsum * ps, (BPC, H)).reshape(-1)

        in_maps.append({
            "encT": encT[sl],
            "wblob": wblob,
            "onesc": onesc,
            "fblob": fblob,
            "r1lhs": r1l,
            "r1rhs": r1r.astype(BF16),
            "vbc": vbc,
        })
    return in_maps


def kernel(dec_input, enc_output, text_lengths, coverage_vector, W, b, v_w, v_b):
    from concourse.bass_utils import run_bass_kernel_spmd

    nc = _get_nc()
    in_maps = _prep_in_maps(dec_input, enc_output, text_lengths,
                            coverage_vector, W, b, v_w)
    res = run_bass_kernel_spmd(nc, in_maps, core_ids=list(range(NCORES)))

    att = np.empty((B, S), np.float32)
    ncov = np.empty((B, S), np.float32)
    for core in range(NCORES):
        r = res.results[core]
        att[core * BPC:(core + 1) * BPC] = \
            r["att_out"].transpose(0, 2, 1).reshape(BPC, S)
        ncov[core * BPC:(core + 1) * BPC] = \
            r["cov_out"].transpose(0, 2, 1).reshape(BPC, S)
    return att, ncov


# revision 12
# speedup vs baseline: 2.2923x; 1.8575x over previous
"""Fused sparse-attention kernel for Trainium2 (8 NeuronCores, data-parallel over batch).

Computation (per batch element b):
    X[s,k]  = enc[b] @ W_enc + dec_proj[b,k] + cov[b,s]*Wcovsum[k] + bias[k]
    T       = tanh(X)
    att[s]  = T @ v_w                      (+ v_b, which cancels in softmax)
    w       = softmax(att masked to s < len[b])
    new_cov = cov + w
Sharding: batch B=32 split 4-per-core across 8 cores; weights replicated.

Key layout/precision choices:
- enc is cast+transposed ON THE HOST to fp8 e4m3 [128p, HC, S] chunk layout, so
  the device does one contiguous full-rate load per batch element (no fp32
  DRAM bounce, no xbar DMA-transpose) and the main GEMM runs fp8 DoubleRow
  (K=256 per pass at 0.5 cyc/row = 2x PE throughput).
- fp8 operands are pre-scaled (enc*0.25, W_enc*16) to dodge e4m3 subnormals;
  the net *4 on psum is undone by the tanh's free scale arg. Host-emulated
  end-to-end relmax vs the fp32 reference: 6.9e-3 (gate 2e-2).
- The additive terms (dec_proj+bias, cov*Wcovsum) stay a bf16 K=2 rank-1
  matmul into the same psum group (R1_FP8 flips them to a K=2 fp8 DoubleRow
  pass at half PE cost, relmax 1.18e-2).
- dec_proj (dec @ W_s, 17 MFLOP total) and Wcovsum are host-computed.

Device pipeline, two s-tiles (=2 psum banks) per step:
  PE:  two accumulation groups into one [128, 2*512] psum pair-tile
  ACT: one tanh over the pair (amortizes the psum-access init cost) -> bf16
  DVE: per s-tile fused T*v multiply + free-dim reduce (stt accum_out)
Masked-softmax tail per batch in [s_lo=128, s_hi=16] layout: exp on ACT,
iota<len mask fused with the exp multiply on DVE, fp32 sum-matmul + 1/sum
broadcast via two tiny PE matmuls (max-subtraction skipped: |logits| <=
||v||_1 ~ 8, safely inside fp32 exp range; v_b cancels in softmax).
DMA order: batch-0 first quarter + the three first-needed consts go first so
the first matmul issues ~3us in; everything else streams behind it.
"""

import numpy as np
import ml_dtypes

B, S, H, E = 32, 2048, 512, 512
NCORES = 8
BPC = B // NCORES           # batches per core
SLO, SHI = 128, S // 128    # att tile layout: s = 128*j + p  ->  [p, j]
HC = H // 128               # h chunks
BF16 = ml_dtypes.bfloat16

USE_FP8 = True
R1_FP8 = False              # rank-1 terms as fp8 DoubleRow (cheaper PE, more err)
FP8 = ml_dtypes.float8_e4m3fn
ENC_SCALE = 0.25            # enc pre-scale (host)
W_SCALE = 16.0              # W_enc pre-scale (host)
PSUM_SCALE = ENC_SCALE * W_SCALE  # net scale on psum; undone in tanh

_CACHE = {}


def _build_nc():
    import concourse.mybir as mybir
    import concourse.tile as tile
    from concourse import bacc
    from contextlib import ExitStack

    dt = mybir.dt
    F32, BF = dt.float32, dt.bfloat16
    ENC_DT = dt.float8e4 if USE_FP8 else BF

    nc = bacc.Bacc("TRN2", target_bir_lowering=False, debug=False,
                   enable_asserts=False, num_devices=NCORES)

    # ---- DRAM I/O (per-core shapes) ----
    # encT[b, p, c*S + s] = enc[b, s, 128c+p]  (pre-scaled when fp8)
    encT = nc.dram_tensor("encT", [BPC, 128, HC * S], ENC_DT,
                          kind="ExternalInput").ap()
    # wblob: wenc chunk c at cols [c*H, (c+1)*H): wenc[c][p, k] = W[128c+p, k]
    wblob = nc.dram_tensor("wblob", [128, HC * H], ENC_DT,
                           kind="ExternalInput").ap()
    # ones col (f32) for the softmax-sum matmul
    onesc = nc.dram_tensor("onesc", [128, 1], F32, kind="ExternalInput").ap()
    # f32 blob: [iota (SHI) | lens (BPC) | ones_row (128, row 0) | cov_t (BPC*SHI)]
    fblob = nc.dram_tensor("fblob", [SLO, SHI + BPC + 128 + BPC * SHI], F32,
                           kind="ExternalInput").ap()
    if R1_FP8:
        r1lhs = nc.dram_tensor("r1lhs", [1, 2 * BPC * S], ENC_DT,
                               kind="ExternalInput").ap()
        r1rhs = nc.dram_tensor("r1rhs", [1, 2 * BPC * H], ENC_DT,
                               kind="ExternalInput").ap()
    else:
        r1lhs = nc.dram_tensor("r1lhs", [2, BPC * S], BF, kind="ExternalInput").ap()
        # r1rhs row0 per batch: (dec_proj[b]+b)*PS ; row1: Wcovsum*PS
        r1rhs = nc.dram_tensor("r1rhs", [2, BPC * H], BF, kind="ExternalInput").ap()
    vbc = nc.dram_tensor("vbc", [128, H], BF, kind="ExternalInput").ap()
    att_out = nc.dram_tensor("att_out", [BPC, SLO, SHI], F32, kind="ExternalOutput").ap()
    cov_out = nc.dram_tensor("cov_out", [BPC, SLO, SHI], F32, kind="ExternalOutput").ap()

    AF = mybir.ActivationFunctionType
    OP = mybir.AluOpType
    DR = mybir.MatmulPerfMode.DoubleRow

    with tile.TileContext(nc) as tc, ExitStack() as ctx:
        consts = ctx.enter_context(tc.tile_pool(name="consts", bufs=1))
        encp = ctx.enter_context(tc.tile_pool(name="encp", bufs=2))
        tpool = ctx.enter_context(tc.tile_pool(name="tpool", bufs=3))
        spool = ctx.enter_context(tc.tile_pool(name="spool", bufs=2))
        small = ctx.enter_context(tc.tile_pool(name="small", bufs=2))
        attp = ctx.enter_context(tc.tile_pool(name="attp", bufs=2))
        ppm = ctx.enter_context(tc.tile_pool(name="ppm", bufs=3, space="PSUM"))
        pps = ctx.enter_context(tc.tile_pool(name="pps", bufs=1, space="PSUM"))

        # ---- DMA order: batch-0 first quarter, then the first-needed consts,
        # then the rest of batch 0, then the remaining consts. Input DMAs ride
        # the SP (sync) queue; DMA_ENGINES serialize roughly in request order
        # so this gets the first matmul issued ~3us in. ----
        def enc_tile():
            return encp.tile([128, HC, S], ENC_DT, tag="enc", name="enc_t")

        def enc_load(e_t, b, lo, hi):
            src = encT[b].rearrange("p (c s) -> p c s", c=HC)
            nc.sync.dma_start(e_t[:, :, lo:hi], src[:, :, lo:hi])

        e0 = enc_tile()
        enc_load(e0, 0, 0, 512)

        if R1_FP8:
            r1lhs_sb = consts.tile([1, 2 * BPC * S], ENC_DT, tag="r1lhs")
            r1rhs_sb = consts.tile([1, 2 * BPC * H], ENC_DT, tag="r1rhs")
        else:
            r1lhs_sb = consts.tile([2, BPC * S], BF, tag="r1lhs")
            r1rhs_sb = consts.tile([2, BPC * H], BF, tag="r1rhs")
        nc.gpsimd.dma_start(r1lhs_sb[:], r1lhs[:])
        nc.gpsimd.dma_start(r1rhs_sb[:], r1rhs[:])
        wb_sb = consts.tile([128, HC * H], ENC_DT, tag="wblob")
        nc.gpsimd.dma_start(wb_sb[:], wblob[:])

        enc_load(e0, 0, 512, 1024)
        enc_load(e0, 0, 1024, 2048)

        vbc_sb = consts.tile([128, H], BF, tag="vbc")
        nc.gpsimd.dma_start(vbc_sb[:], vbc[:])
        fb_sb = consts.tile([SLO, SHI + BPC + 128 + BPC * SHI], F32, tag="fblob")
        nc.gpsimd.dma_start(fb_sb[:], fblob[:])
        onesc_sb = consts.tile([128, 1], F32, tag="onesc")
        nc.gpsimd.dma_start(onesc_sb[:], onesc[:])

        iota_sb = fb_sb[:, 0:SHI]
        lens_sb = fb_sb[:, SHI:SHI + BPC]
        ones_r_sb = fb_sb[0:1, SHI + BPC:SHI + BPC + 128]
        covt_sb = fb_sb[:, SHI + BPC + 128:]

        def load_batch(b):
            e_t = enc_tile()
            enc_load(e_t, b, 0, 1024)
            enc_load(e_t, b, 1024, 2048)
            return e_t

        pre = {0: e0}
        wb3 = wb_sb[:].rearrange("p (c k) -> p c k", c=HC)
        if R1_FP8:
            r1l3 = r1lhs_sb[:].rearrange("p (x c) -> p x c", x=2)
            r1r3 = r1rhs_sb[:].rearrange("p (x c) -> p x c", x=2)

        # ---- main loop: two s-tiles (2 psum banks) per step ----
        for b in range(BPC):
            enc_t = pre.pop(b)
            if b + 1 < BPC:
                pre[b + 1] = load_batch(b + 1)

            att_t = attp.tile([SLO, SHI], F32, tag="att")
            for j0 in range(0, SHI, 2):
                ps = ppm.tile([128, 2 * H], F32, tag="x")
                for jj in range(2):
                    j = j0 + jj
                    psl = ps[:, jj * H:(jj + 1) * H]
                    if R1_FP8:
                        nc.tensor.matmul(
                            psl,
                            r1l3[:, :, b * S + j * 128: b * S + (j + 1) * 128],
                            r1r3[:, :, b * H:(b + 1) * H],
                            start=True, stop=False, perf_mode=DR,
                        )
                    else:
                        nc.tensor.matmul(
                            psl,
                            r1lhs_sb[:, b * S + j * 128: b * S + (j + 1) * 128],
                            r1rhs_sb[:, b * H:(b + 1) * H],
                            start=True, stop=False,
                        )
                    if USE_FP8:
                        for c in range(0, HC, 2):
                            nc.tensor.matmul(
                                psl,
                                enc_t[:, c:c + 2, j * 128:(j + 1) * 128],
                                wb3[:, c:c + 2, :],
                                start=False, stop=(c + 2 == HC),
                                perf_mode=DR,
                            )
                    else:
                        for c in range(HC):
                            nc.tensor.matmul(
                                psl,
                                enc_t[:, c, j * 128:(j + 1) * 128],
                                wb3[:, c, :],
                                start=False, stop=(c == HC - 1),
                            )
                t_t = tpool.tile([128, 2 * H], BF, tag="t")
                nc.scalar.activation(t_t[:], ps[:], AF.Tanh,
                                     scale=1.0 / PSUM_SCALE if USE_FP8 else 1.0)
                for jj in range(2):
                    j = j0 + jj
                    scr = spool.tile([128, H], BF, tag="scr")
                    nc.vector.scalar_tensor_tensor(
                        out=scr[:], in0=t_t[:, jj * H:(jj + 1) * H],
                        scalar=1.0, in1=vbc_sb[:],
                        op0=OP.mult, op1=OP.mult,
                        accum_out=att_t[:, j:j + 1],
                    )

            # ---- masked softmax tail (tiny) ----
            expt = small.tile([SLO, SHI], F32, tag="expt")
            nc.scalar.activation(expt[:], att_t[:], AF.Exp)
            mexp = small.tile([SLO, SHI], F32, tag="mexp")
            nc.vector.scalar_tensor_tensor(
                out=mexp[:], in0=iota_sb, scalar=lens_sb[:, b:b + 1],
                in1=expt[:], op0=OP.is_lt, op1=OP.mult,
            )
            sum_ps = pps.tile([1, SHI], F32, tag="sum")
            nc.tensor.matmul(sum_ps[:], onesc_sb[:], mexp[:],
                             start=True, stop=True)
            ssum = small.tile([1, 1], F32, tag="ssum")
            nc.vector.reduce_sum(ssum[:], sum_ps[:], axis=mybir.AxisListType.X)
            sinv = small.tile([1, 1], F32, tag="sinv")
            nc.vector.reciprocal(sinv[:], ssum[:])
            inv_ps = pps.tile([128, 1], F32, tag="inv")
            nc.tensor.matmul(inv_ps[:], ones_r_sb, sinv[:], start=True, stop=True)
            wts = small.tile([SLO, SHI], F32, tag="wts")
            nc.vector.tensor_scalar(wts[:], mexp[:], inv_ps[:], None, OP.mult)
            nc.sync.dma_start(att_out[b], wts[:])
            ncov = small.tile([SLO, SHI], F32, tag="ncov")
            nc.vector.tensor_tensor(ncov[:], wts[:],
                                    covt_sb[:, b * SHI:(b + 1) * SHI], OP.add)
            nc.sync.dma_start(cov_out[b], ncov[:])

    nc.compile()
    return nc


def _get_nc():
    if "nc" not in _CACHE:
        _CACHE["nc"] = _build_nc()
    return _CACHE["nc"]


def _prep_in_maps(dec_input, enc_output, text_lengths, coverage_vector, W, b, v_w):
    enc = np.asarray(enc_output, dtype=np.float32)
    dec = np.asarray(dec_input, dtype=np.float32).reshape(B, E)
    cov = np.asarray(coverage_vector, dtype=np.float32)
    W = np.asarray(W, dtype=np.float32)
    b = np.asarray(b, dtype=np.float32)
    v_w = np.asarray(v_w, dtype=np.float32)
    lens_f = np.asarray(text_lengths).astype(np.float32)

    enc_dt = FP8 if USE_FP8 else BF16
    ps = PSUM_SCALE if USE_FP8 else 1.0
    es = ENC_SCALE if USE_FP8 else 1.0
    ws = W_SCALE if USE_FP8 else 1.0

    # enc^T chunk layout [B, 128, HC, S], host-cast (+pre-scale for fp8)
    encT = (enc * es if USE_FP8 else enc).reshape(B, S, HC, 128) \
        .transpose(0, 3, 2, 1)
    encT = np.ascontiguousarray(encT).astype(enc_dt).reshape(B, 128, HC * S)

    wenc = W[:H] * ws                                  # (H, H)
    wblob = np.ascontiguousarray(
        wenc.reshape(HC, 128, H).transpose(1, 0, 2).reshape(128, HC * H)
    ).astype(enc_dt)

    dec_proj = dec @ W[H:H + E] + b                    # (B, H)
    wcovsum = W[H + E:].sum(axis=0, dtype=np.float32)  # (H,)

    vbc = np.ascontiguousarray(np.broadcast_to(v_w.astype(BF16), (128, H)))
    onesc = np.ones((128, 1), np.float32)
    iota = (np.arange(SLO, dtype=np.float32)[:, None]
            + 128.0 * np.arange(SHI, dtype=np.float32)[None, :])

    in_maps = []
    for core in range(NCORES):
        sl = slice(core * BPC, (core + 1) * BPC)

        fblob = np.empty((SLO, SHI + BPC + 128 + BPC * SHI), np.float32)
        fblob[:, 0:SHI] = iota
        fblob[:, SHI:SHI + BPC] = lens_f[sl][None, :]
        fblob[:, SHI + BPC:SHI + BPC + 128] = 1.0
        fblob[:, SHI + BPC + 128:] = (cov[sl].reshape(BPC, SHI, SLO)
                                      .transpose(2, 0, 1).reshape(SLO, BPC * SHI))

        if R1_FP8:
            r1l = np.empty((1, 2, BPC * S), np.float32)
            r1l[0, 0] = 1.0
            r1l[0, 1] = cov[sl].reshape(-1)
            r1r = np.empty((1, 2, BPC * H), np.float32)
            r1r[0, 0] = (dec_proj[sl] * ps).reshape(-1)
            r1r[0, 1] = np.broadcast_to(wcovsum * ps, (BPC, H)).reshape(-1)
            r1l = r1l.reshape(1, 2 * BPC * S).astype(enc_dt)
            r1r = r1r.reshape(1, 2 * BPC * H).astype(enc_dt)
        else:
            r1l = np.empty((2, BPC * S), BF16)
            r1l[0] = BF16(1.0)
            r1l[1] = cov[sl].astype(BF16).reshape(-1)
            r1r = np.empty((2, BPC * H), np.float32)
            r1r[0] = (dec_proj[sl] * ps).reshape(-1)
            r1r[1] = np.broadcast_to(wcovsum * ps, (BPC, H)).reshape(-1)
            r1r = r1r.astype(BF16)

        in_maps.append({
            "encT": encT[sl],
            "wblob": wblob,
            "onesc": onesc,
            "fblob": fblob,
            "r1lhs": r1l,
            "r1rhs": r1r,
            "vbc": vbc,
        })
    return in_maps


def kernel(dec_input, enc_output, text_lengths, coverage_vector, W, b, v_w, v_b):
    from concourse.bass_utils import run_bass_kernel_spmd

    nc = _get_nc()
    in_maps = _prep_in_maps(dec_input, enc_output, text_lengths,
                            coverage_vector, W, b, v_w)
    res = run_bass_kernel_spmd(nc, in_maps, core_ids=list(range(NCORES)))

    att = np.empty((B, S), np.float32)
    ncov = np.empty((B, S), np.float32)
    for core in range(NCORES):
        r = res.results[core]
        att[core * BPC:(core + 1) * BPC] = \
            r["att_out"].transpose(0, 2, 1).reshape(BPC, S)
        ncov[core * BPC:(core + 1) * BPC] = \
            r["cov_out"].transpose(0, 2, 1).reshape(BPC, S)
    return att, ncov


# revision 19
# speedup vs baseline: 2.5245x; 1.1013x over previous
"""Fused sparse-attention kernel for Trainium2 (8 NeuronCores, data-parallel over batch).

Computation (per batch element b):
    X[s,k]  = enc[b] @ W_enc + dec_proj[b,k] + cov[b,s]*Wcovsum[k] + bias[k]
    T       = tanh(X)
    att[s]  = T @ v_w                      (+ v_b, which cancels in softmax)
    w       = softmax(att masked to s < len[b])
    new_cov = cov + w
Sharding: batch B=32 split 4-per-core across 8 cores; weights replicated.

Key layout/precision choices:
- enc is cast+transposed ON THE HOST to fp8 e4m3 [128p, HC, S] chunk layout, so
  the device does one contiguous full-rate load per batch element (no fp32
  DRAM bounce, no xbar DMA-transpose) and the main GEMM runs fp8 DoubleRow
  (K=256 per pass at 0.5 cyc/row = 2x PE throughput).
- fp8 operands are pre-scaled (enc*0.25, W_enc*16) to dodge e4m3 subnormals;
  the net *4 on psum is undone by the tanh's free scale arg. Host-emulated
  end-to-end relmax vs the fp32 reference: 6.9e-3 (gate 2e-2).
- The additive terms (dec_proj+bias, cov*Wcovsum) stay a bf16 K=2 rank-1
  matmul into the same psum group (R1_FP8 flips them to a K=2 fp8 DoubleRow
  pass at half PE cost, relmax 1.18e-2).
- dec_proj (dec @ W_s, 17 MFLOP total) and Wcovsum are host-computed.

Device pipeline, two s-tiles (=2 psum banks) per step:
  PE:  two accumulation groups into one [128, 2*512] psum pair-tile
  ACT: one tanh over the pair (amortizes the psum-access init cost) -> bf16
  DVE: one paired tensor_tensor T*v multiply (2x bf16 mode), then per s-tile
       tensor_scalar with accum_out for the free-dim reduce (4x mode). The
       obvious single scalar_tensor_tensor runs at 1x (no DVE perf-mode uop),
       so this 3-op split is ~20% faster overall.
Masked-softmax tail per batch in [s_lo=128, s_hi=16] layout: exp on ACT,
iota<len mask fused with the exp multiply on DVE, fp32 sum-matmul + 1/sum
broadcast via two tiny PE matmuls (max-subtraction skipped: |logits| <=
||v||_1 ~ 8, safely inside fp32 exp range; v_b cancels in softmax).
DMA order: batch-0 first quarter + the three first-needed consts go first so
the first matmul issues ~3us in; everything else streams behind it.
"""

import numpy as np
import ml_dtypes

B, S, H, E = 32, 2048, 512, 512
NCORES = 8
BPC = B // NCORES           # batches per core
SLO, SHI = 128, S // 128    # att tile layout: s = 128*j + p  ->  [p, j]
HC = H // 128               # h chunks
BF16 = ml_dtypes.bfloat16

USE_FP8 = True
R1_FP8 = False              # rank-1 terms as fp8 DoubleRow (cheaper PE, more err)
FP8 = ml_dtypes.float8_e4m3fn
ENC_SCALE = 0.25            # enc pre-scale (host)
W_SCALE = 16.0              # W_enc pre-scale (host)
PSUM_SCALE = ENC_SCALE * W_SCALE  # net scale on psum; undone in tanh

_CACHE = {}


def _build_nc():
    import concourse.mybir as mybir
    import concourse.tile as tile
    from concourse import bacc
    from contextlib import ExitStack

    dt = mybir.dt
    F32, BF = dt.float32, dt.bfloat16
    ENC_DT = dt.float8e4 if USE_FP8 else BF

    nc = bacc.Bacc("TRN2", target_bir_lowering=False, debug=False,
                   enable_asserts=False, num_devices=NCORES)

    # ---- DRAM I/O (per-core shapes) ----
    # encT[b, p, c*S + s] = enc[b, s, 128c+p]  (pre-scaled when fp8)
    encT = nc.dram_tensor("encT", [BPC, 128, HC * S], ENC_DT,
                          kind="ExternalInput").ap()
    # wblob: wenc chunk c at cols [c*H, (c+1)*H): wenc[c][p, k] = W[128c+p, k]
    wblob = nc.dram_tensor("wblob", [128, HC * H], ENC_DT,
                           kind="ExternalInput").ap()
    # ones col (f32) for the softmax-sum matmul
    onesc = nc.dram_tensor("onesc", [128, 1], F32, kind="ExternalInput").ap()
    # f32 blob: [iota (SHI) | lens (BPC) | ones_row (128, row 0) | cov_t (BPC*SHI)]
    fblob = nc.dram_tensor("fblob", [SLO, SHI + BPC + 128 + BPC * SHI], F32,
                           kind="ExternalInput").ap()
    if R1_FP8:
        r1lhs = nc.dram_tensor("r1lhs", [1, 2 * BPC * S], ENC_DT,
                               kind="ExternalInput").ap()
        r1rhs = nc.dram_tensor("r1rhs", [1, 2 * BPC * H], ENC_DT,
                               kind="ExternalInput").ap()
    else:
        r1lhs = nc.dram_tensor("r1lhs", [2, BPC * S], BF, kind="ExternalInput").ap()
        # r1rhs row0 per batch: (dec_proj[b]+b)*PS ; row1: Wcovsum*PS
        r1rhs = nc.dram_tensor("r1rhs", [2, BPC * H], BF, kind="ExternalInput").ap()
    vbc = nc.dram_tensor("vbc", [128, 2 * H], BF, kind="ExternalInput").ap()
    att_out = nc.dram_tensor("att_out", [BPC, SLO, SHI], F32, kind="ExternalOutput").ap()
    cov_out = nc.dram_tensor("cov_out", [BPC, SLO, SHI], F32, kind="ExternalOutput").ap()

    AF = mybir.ActivationFunctionType
    OP = mybir.AluOpType
    DR = mybir.MatmulPerfMode.DoubleRow

    with tile.TileContext(nc) as tc, ExitStack() as ctx:
        consts = ctx.enter_context(tc.tile_pool(name="consts", bufs=1))
        encp = ctx.enter_context(tc.tile_pool(name="encp", bufs=2))
        tpool = ctx.enter_context(tc.tile_pool(name="tpool", bufs=3))
        spool = ctx.enter_context(tc.tile_pool(name="spool", bufs=2))
        small = ctx.enter_context(tc.tile_pool(name="small", bufs=2))
        attp = ctx.enter_context(tc.tile_pool(name="attp", bufs=2))
        ppm = ctx.enter_context(tc.tile_pool(name="ppm", bufs=3, space="PSUM"))
        pps = ctx.enter_context(tc.tile_pool(name="pps", bufs=1, space="PSUM"))

        # ---- DMA order: batch-0 first quarter, then the first-needed consts,
        # then the rest of batch 0, then the remaining consts. Input DMAs ride
        # the SP (sync) queue; DMA_ENGINES serialize roughly in request order
        # so this gets the first matmul issued ~3us in. ----
        def enc_tile():
            return encp.tile([128, HC, S], ENC_DT, tag="enc", name="enc_t")

        def enc_load(e_t, b, lo, hi):
            src = encT[b].rearrange("p (c s) -> p c s", c=HC)
            nc.sync.dma_start(e_t[:, :, lo:hi], src[:, :, lo:hi])

        e0 = enc_tile()
        enc_load(e0, 0, 0, 256)

        # first-needed consts ride HWDGE queues (SP/ACT/DVE issue; much lower
        # fixed latency than the Pool SWDGE path) so the first matmul can go
        # ~3us in; bulk/later consts go via Pool SWDGE behind them.
        if R1_FP8:
            r1lhs_sb = consts.tile([1, 2 * BPC * S], ENC_DT, tag="r1lhs")
            r1rhs_sb = consts.tile([1, 2 * BPC * H], ENC_DT, tag="r1rhs")
        else:
            r1lhs_sb = consts.tile([2, BPC * S], BF, tag="r1lhs")
            r1rhs_sb = consts.tile([2, BPC * H], BF, tag="r1rhs")
        nc.scalar.dma_start(r1lhs_sb[:], r1lhs[:])
        nc.scalar.dma_start(r1rhs_sb[:], r1rhs[:])
        wb_sb = consts.tile([128, HC * H], ENC_DT, tag="wblob")
        nc.scalar.dma_start(wb_sb[:], wblob[:])

        enc_load(e0, 0, 256, 1024)
        enc_load(e0, 0, 1024, 2048)

        vbc_sb = consts.tile([128, 2 * H], BF, tag="vbc")
        nc.gpsimd.dma_start(vbc_sb[:], vbc[:])
        fb_sb = consts.tile([SLO, SHI + BPC + 128 + BPC * SHI], F32, tag="fblob")
        nc.gpsimd.dma_start(fb_sb[:], fblob[:])
        onesc_sb = consts.tile([128, 1], F32, tag="onesc")
        nc.gpsimd.dma_start(onesc_sb[:], onesc[:])

        iota_sb = fb_sb[:, 0:SHI]
        lens_sb = fb_sb[:, SHI:SHI + BPC]
        ones_r_sb = fb_sb[0:1, SHI + BPC:SHI + BPC + 128]
        covt_sb = fb_sb[:, SHI + BPC + 128:]

        def load_batch(b):
            e_t = enc_tile()
            enc_load(e_t, b, 0, 1024)
            enc_load(e_t, b, 1024, 2048)
            return e_t

        pre = {0: e0}
        wb3 = wb_sb[:].rearrange("p (c k) -> p c k", c=HC)
        if R1_FP8:
            r1l3 = r1lhs_sb[:].rearrange("p (x c) -> p x c", x=2)
            r1r3 = r1rhs_sb[:].rearrange("p (x c) -> p x c", x=2)

        # ---- main loop: two s-tiles (2 psum banks) per step ----
        for b in range(BPC):
            enc_t = pre.pop(b)
            if b + 1 < BPC:
                pre[b + 1] = load_batch(b + 1)

            att_t = attp.tile([SLO, SHI], F32, tag="att")
            for j0 in range(0, SHI, 2):
                ps = ppm.tile([128, 2 * H], F32, tag="x")
                for jj in range(2):
                    j = j0 + jj
                    psl = ps[:, jj * H:(jj + 1) * H]
                    if R1_FP8:
                        nc.tensor.matmul(
                            psl,
                            r1l3[:, :, b * S + j * 128: b * S + (j + 1) * 128],
                            r1r3[:, :, b * H:(b + 1) * H],
                            start=True, stop=False, perf_mode=DR,
                        )
                    else:
                        nc.tensor.matmul(
                            psl,
                            r1lhs_sb[:, b * S + j * 128: b * S + (j + 1) * 128],
                            r1rhs_sb[:, b * H:(b + 1) * H],
                            start=True, stop=False,
                        )
                    if USE_FP8:
                        for c in range(0, HC, 2):
                            nc.tensor.matmul(
                                psl,
                                enc_t[:, c:c + 2, j * 128:(j + 1) * 128],
                                wb3[:, c:c + 2, :],
                                start=False, stop=(c + 2 == HC),
                                perf_mode=DR,
                            )
                    else:
                        for c in range(HC):
                            nc.tensor.matmul(
                                psl,
                                enc_t[:, c, j * 128:(j + 1) * 128],
                                wb3[:, c, :],
                                start=False, stop=(c == HC - 1),
                            )
                t_t = tpool.tile([128, 2 * H], BF, tag="t")
                nc.scalar.activation(t_t[:], ps[:], AF.Tanh,
                                     scale=1.0 / PSUM_SCALE if USE_FP8 else 1.0)
                scr = spool.tile([128, 2 * H], BF, tag="scr")
                nc.vector.tensor_tensor(scr[:], t_t[:], vbc_sb[:], OP.mult)
                for jj in range(2):
                    j = j0 + jj
                    scr2 = spool.tile([128, H], BF, tag="scr2")
                    nc.vector.tensor_scalar(
                        scr2[:], scr[:, jj * H:(jj + 1) * H], 1.0, None,
                        OP.mult, OP.add, accum_out=att_t[:, j:j + 1],
                    )

            # ---- masked softmax tail (tiny) ----
            expt = small.tile([SLO, SHI], F32, tag="expt")
            nc.scalar.activation(expt[:], att_t[:], AF.Exp)
            mexp = small.tile([SLO, SHI], F32, tag="mexp")
            nc.vector.scalar_tensor_tensor(
                out=mexp[:], in0=iota_sb, scalar=lens_sb[:, b:b + 1],
                in1=expt[:], op0=OP.is_lt, op1=OP.mult,
            )
            sum_ps = pps.tile([1, SHI], F32, tag="sum")
            nc.tensor.matmul(sum_ps[:], onesc_sb[:], mexp[:],
                             start=True, stop=True)
            ssum = small.tile([1, 1], F32, tag="ssum")
            nc.vector.reduce_sum(ssum[:], sum_ps[:], axis=mybir.AxisListType.X)
            sinv = small.tile([1, 1], F32, tag="sinv")
            nc.vector.reciprocal(sinv[:], ssum[:])
            inv_ps = pps.tile([128, 1], F32, tag="inv")
            nc.tensor.matmul(inv_ps[:], ones_r_sb, sinv[:], start=True, stop=True)
            wts = small.tile([SLO, SHI], F32, tag="wts")
            nc.vector.tensor_scalar(wts[:], mexp[:], inv_ps[:], None, OP.mult)
            nc.sync.dma_start(att_out[b], wts[:])
            ncov = small.tile([SLO, SHI], F32, tag="ncov")
            nc.vector.tensor_tensor(ncov[:], wts[:],
                                    covt_sb[:, b * SHI:(b + 1) * SHI], OP.add)
            nc.sync.dma_start(cov_out[b], ncov[:])

    nc.compile()
    return nc


def _get_nc():
    if "nc" not in _CACHE:
        _CACHE["nc"] = _build_nc()
    return _CACHE["nc"]


def _prep_in_maps(dec_input, enc_output, text_lengths, coverage_vector, W, b, v_w):
    enc = np.asarray(enc_output, dtype=np.float32)
    dec = np.asarray(dec_input, dtype=np.float32).reshape(B, E)
    cov = np.asarray(coverage_vector, dtype=np.float32)
    W = np.asarray(W, dtype=np.float32)
    b = np.asarray(b, dtype=np.float32)
    v_w = np.asarray(v_w, dtype=np.float32)
    lens_f = np.asarray(text_lengths).astype(np.float32)

    enc_dt = FP8 if USE_FP8 else BF16
    ps = PSUM_SCALE if USE_FP8 else 1.0
    es = ENC_SCALE if USE_FP8 else 1.0
    ws = W_SCALE if USE_FP8 else 1.0

    # enc^T chunk layout [B, 128, HC, S], host-cast (+pre-scale for fp8)
    encT = (enc * es if USE_FP8 else enc).reshape(B, S, HC, 128) \
        .transpose(0, 3, 2, 1)
    encT = np.ascontiguousarray(encT).astype(enc_dt).reshape(B, 128, HC * S)

    wenc = W[:H] * ws                                  # (H, H)
    wblob = np.ascontiguousarray(
        wenc.reshape(HC, 128, H).transpose(1, 0, 2).reshape(128, HC * H)
    ).astype(enc_dt)

    dec_proj = dec @ W[H:H + E] + b                    # (B, H)
    wcovsum = W[H + E:].sum(axis=0, dtype=np.float32)  # (H,)

    vbc = np.ascontiguousarray(np.broadcast_to(
        np.concatenate([v_w, v_w]).astype(BF16), (128, 2 * H)))
    onesc = np.ones((128, 1), np.float32)
    iota = (np.arange(SLO, dtype=np.float32)[:, None]
            + 128.0 * np.arange(SHI, dtype=np.float32)[None, :])

    in_maps = []
    for core in range(NCORES):
        sl = slice(core * BPC, (core + 1) * BPC)

        fblob = np.empty((SLO, SHI + BPC + 128 + BPC * SHI), np.float32)
        fblob[:, 0:SHI] = iota
        fblob[:, SHI:SHI + BPC] = lens_f[sl][None, :]
        fblob[:, SHI + BPC:SHI + BPC + 128] = 1.0
        fblob[:, SHI + BPC + 128:] = (cov[sl].reshape(BPC, SHI, SLO)
                                      .transpose(2, 0, 1).reshape(SLO, BPC * SHI))

        if R1_FP8:
            r1l = np.empty((1, 2, BPC * S), np.float32)
            r1l[0, 0] = 1.0
            r1l[0, 1] = cov[sl].reshape(-1)
            r1r = np.empty((1, 2, BPC * H), np.float32)
            r1r[0, 0] = (dec_proj[sl] * ps).reshape(-1)
            r1r[0, 1] = np.broadcast_to(wcovsum * ps, (BPC, H)).reshape(-1)
            r1l = r1l.reshape(1, 2 * BPC * S).astype(enc_dt)
            r1r = r1r.reshape(1, 2 * BPC * H).astype(enc_dt)
        else:
            r1l = np.empty((2, BPC * S), BF16)
            r1l[0] = BF16(1.0)
            r1l[1] = cov[sl].astype(BF16).reshape(-1)
            r1r = np.empty((2, BPC * H), np.float32)
            r1r[0] = (dec_proj[sl] * ps).reshape(-1)
            r1r[1] = np.broadcast_to(wcovsum * ps, (BPC, H)).reshape(-1)
            r1r = r1r.astype(BF16)

        in_maps.append({
            "encT": encT[sl],
            "wblob": wblob,
            "onesc": onesc,
            "fblob": fblob,
            "r1lhs": r1l,
            "r1rhs": r1r,
            "vbc": vbc,
        })
    return in_maps


def kernel(dec_input, enc_output, text_lengths, coverage_vector, W, b, v_w, v_b):
    from concourse.bass_utils import run_bass_kernel_spmd

    nc = _get_nc()
    in_maps = _prep_in_maps(dec_input, enc_output, text_lengths,
                            coverage_vector, W, b, v_w)
    res = run_bass_kernel_spmd(nc, in_maps, core_ids=list(range(NCORES)))

    att = np.empty((B, S), np.float32)
    ncov = np.empty((B, S), np.float32)
    for core in range(NCORES):
        r = res.results[core]
        att[core * BPC:(core + 1) * BPC] = \
            r["att_out"].transpose(0, 2, 1).reshape(BPC, S)
        ncov[core * BPC:(core + 1) * BPC] = \
            r["cov_out"].transpose(0, 2, 1).reshape(BPC, S)
    return att, ncov


# revision 29
# speedup vs baseline: 2.5661x; 1.0165x over previous
"""Fused sparse-attention kernel for Trainium2 (8 NeuronCores, data-parallel over batch).

Computation (per batch element b):
    X[s,k]  = enc[b] @ W_enc + dec_proj[b,k] + cov[b,s]*Wcovsum[k] + bias[k]
    T       = tanh(X)
    att[s]  = T @ v_w                      (+ v_b, which cancels in softmax)
    w       = softmax(att masked to s < len[b])
    new_cov = cov + w
Sharding: batch B=32 split 4-per-core across 8 cores; weights replicated.

Key layout/precision choices:
- enc is cast+transposed ON THE HOST to fp8 e4m3 [128p, HC, S] chunk layout, so
  the device does one contiguous full-rate load per batch element (no fp32
  DRAM bounce, no xbar DMA-transpose) and the main GEMM runs fp8 DoubleRow
  (K=256 per pass at 0.5 cyc/row = 2x PE throughput).
- fp8 operands are pre-scaled (enc*0.25, W_enc*16) to dodge e4m3 subnormals;
  the net *4 on psum is undone by the tanh's free scale arg. Host-emulated
  end-to-end relmax vs the fp32 reference: 6.9e-3 (gate 2e-2).
- The additive terms (dec_proj+bias, cov*Wcovsum) stay a bf16 K=2 rank-1
  matmul into the same psum group (R1_FP8 flips them to a K=2 fp8 DoubleRow
  pass at half PE cost, relmax 1.18e-2).
- dec_proj (dec @ W_s, 17 MFLOP total) and Wcovsum are host-computed.

Device pipeline, two s-tiles (=2 psum banks) per step:
  PE:  two accumulation groups into one [128, 2*512] psum pair-tile
  ACT: one tanh over the pair (amortizes the psum-access init cost) -> bf16
  DVE: one paired tensor_tensor T*v multiply (2x bf16 mode), then per s-tile
       tensor_scalar with accum_out for the free-dim reduce (4x mode). The
       obvious single scalar_tensor_tensor runs at 1x (no DVE perf-mode uop),
       so this 3-op split is ~20% faster overall.
Masked-softmax tail per batch in [s_lo=128, s_hi=16] layout: exp on ACT,
iota<len mask fused with the exp multiply on DVE, fp32 sum-matmul + 1/sum
broadcast via two tiny PE matmuls (max-subtraction skipped: |logits| <=
||v||_1 ~ 8, safely inside fp32 exp range; v_b cancels in softmax).
DMA order: batch-0 first quarter + the three first-needed consts go first so
the first matmul issues ~3us in; everything else streams behind it.
"""

import numpy as np
import ml_dtypes

B, S, H, E = 32, 2048, 512, 512
NCORES = 8
BPC = B // NCORES           # batches per core
SLO, SHI = 128, S // 128    # att tile layout: s = 128*j + p  ->  [p, j]
HC = H // 128               # h chunks
BF16 = ml_dtypes.bfloat16

USE_FP8 = True
R1_FP8 = False              # rank-1 terms as fp8 DoubleRow (cheaper PE, more err)
FP8 = ml_dtypes.float8_e4m3fn
ENC_SCALE = 0.25            # enc pre-scale (host)
W_SCALE = 16.0              # W_enc pre-scale (host)
PSUM_SCALE = ENC_SCALE * W_SCALE  # net scale on psum; undone in tanh

_CACHE = {}


def _build_nc():
    import concourse.mybir as mybir
    import concourse.tile as tile
    from concourse import bacc
    from contextlib import ExitStack

    dt = mybir.dt
    F32, BF = dt.float32, dt.bfloat16
    ENC_DT = dt.float8e4 if USE_FP8 else BF

    nc = bacc.Bacc("TRN2", target_bir_lowering=False, debug=False,
                   enable_asserts=False, num_devices=NCORES)

    # ---- DRAM I/O (per-core shapes) ----
    # encT[b, p, c*S + s] = enc[b, s, 128c+p]  (pre-scaled when fp8)
    encT = nc.dram_tensor("encT", [BPC, 128, HC * S], ENC_DT,
                          kind="ExternalInput").ap()
    # wblob: wenc chunk c at cols [c*H, (c+1)*H): wenc[c][p, k] = W[128c+p, k]
    wblob = nc.dram_tensor("wblob", [128, HC * H], ENC_DT,
                           kind="ExternalInput").ap()
    # ones col (f32) for the softmax-sum matmul
    onesc = nc.dram_tensor("onesc", [128, 1], F32, kind="ExternalInput").ap()
    # f32 blob: [iota (SHI) | lens (BPC)]
    fblob = nc.dram_tensor("fblob", [SLO, SHI + BPC], F32,
                           kind="ExternalInput").ap()
    if R1_FP8:
        r1 = nc.dram_tensor("r1", [1, 2 * BPC * (S + H)], ENC_DT,
                            kind="ExternalInput").ap()
    else:
        # [lhs (ones,cov) BPC*S | rhs ((dec_proj+b)*PS, Wcovsum*PS) BPC*H]
        r1 = nc.dram_tensor("r1", [2, BPC * (S + H)], BF,
                            kind="ExternalInput").ap()
    vbc = nc.dram_tensor("vbc", [128, 2 * H], BF, kind="ExternalInput").ap()
    # unnormalized masked exp(att) + its sums; host divides (exact) + cov add
    att_out = nc.dram_tensor("att_out", [BPC, SLO, SHI], F32, kind="ExternalOutput").ap()
    sum_out = nc.dram_tensor("sum_out", [BPC, 1], F32, kind="ExternalOutput").ap()

    AF = mybir.ActivationFunctionType
    OP = mybir.AluOpType
    DR = mybir.MatmulPerfMode.DoubleRow

    with tile.TileContext(nc) as tc, ExitStack() as ctx:
        consts = ctx.enter_context(tc.tile_pool(name="consts", bufs=1))
        encp = ctx.enter_context(tc.tile_pool(name="encp", bufs=2))
        tpool = ctx.enter_context(tc.tile_pool(name="tpool", bufs=3))
        spool = ctx.enter_context(tc.tile_pool(name="spool", bufs=2))
        small = ctx.enter_context(tc.tile_pool(name="small", bufs=2))
        attp = ctx.enter_context(tc.tile_pool(name="attp", bufs=2))
        ppm = ctx.enter_context(tc.tile_pool(name="ppm", bufs=3, space="PSUM"))
        pps = ctx.enter_context(tc.tile_pool(name="pps", bufs=1, space="PSUM"))

        # ---- DMA order: batch-0 first quarter, then the first-needed consts,
        # then the rest of batch 0, then the remaining consts. Input DMAs ride
        # the SP (sync) queue; DMA_ENGINES serialize roughly in request order
        # so this gets the first matmul issued ~3us in. ----
        def enc_tile():
            return encp.tile([128, HC, S], ENC_DT, tag="enc", name="enc_t")

        def enc_load(e_t, b, lo, hi):
            src = encT[b].rearrange("p (c s) -> p c s", c=HC)
            nc.sync.dma_start(e_t[:, :, lo:hi], src[:, :, lo:hi])

        e0 = enc_tile()
        enc_load(e0, 0, 0, 256)

        # first-needed consts ride the ACT HWDGE queue (much lower fixed
        # latency than the Pool SWDGE path) so the first matmul can go ~3us
        # in; bulk/later consts go via Pool SWDGE behind them.
        if R1_FP8:
            r1_sb = consts.tile([1, 2 * BPC * (S + H)], ENC_DT, tag="r1")
        else:
            r1_sb = consts.tile([2, BPC * (S + H)], BF, tag="r1")
        nc.scalar.dma_start(r1_sb[:], r1[:])
        wb_sb = consts.tile([128, HC * H], ENC_DT, tag="wblob")
        nc.scalar.dma_start(wb_sb[:], wblob[:])
        if R1_FP8:
            r1l3 = r1_sb[:, 0:2 * BPC * S].rearrange("p (x c) -> p x c", x=2)
            r1r3 = r1_sb[:, 2 * BPC * S:].rearrange("p (x c) -> p x c", x=2)
        else:
            r1lhs_sb = r1_sb[:, 0:BPC * S]
            r1rhs_sb = r1_sb[:, BPC * S:]

        enc_load(e0, 0, 256, 1024)
        enc_load(e0, 0, 1024, 2048)

        vbc_sb = consts.tile([128, 2 * H], BF, tag="vbc")
        nc.gpsimd.dma_start(vbc_sb[:], vbc[:])
        fb_sb = consts.tile([SLO, SHI + BPC], F32, tag="fblob")
        nc.gpsimd.dma_start(fb_sb[:], fblob[:])
        onesc_sb = consts.tile([128, 1], F32, tag="onesc")
        nc.gpsimd.dma_start(onesc_sb[:], onesc[:])

        iota_sb = fb_sb[:, 0:SHI]
        lens_sb = fb_sb[:, SHI:SHI + BPC]

        def load_batch(b):
            e_t = enc_tile()
            enc_load(e_t, b, 0, 1024)
            enc_load(e_t, b, 1024, 2048)
            return e_t

        pre = {0: e0}
        wb3 = wb_sb[:].rearrange("p (c k) -> p c k", c=HC)

        # ---- main loop: two s-tiles (2 psum banks) per step ----
        for b in range(BPC):
            enc_t = pre.pop(b)
            if b + 1 < BPC:
                pre[b + 1] = load_batch(b + 1)

            att_t = attp.tile([SLO, SHI], F32, tag="att")
            for j0 in range(0, SHI, 2):
                # the very first pair runs as two single-bank groups so the
                # first tanh issues one group earlier (shorter head latency)
                split_first = b == 0 and j0 == 0
                ps = ppm.tile([128, 2 * H], F32, tag="x")
                for jj in range(2):
                    j = j0 + jj
                    psl = ps[:, jj * H:(jj + 1) * H]
                    if R1_FP8:
                        nc.tensor.matmul(
                            psl,
                            r1l3[:, :, b * S + j * 128: b * S + (j + 1) * 128],
                            r1r3[:, :, b * H:(b + 1) * H],
                            start=True, stop=False, perf_mode=DR,
                        )
                    else:
                        nc.tensor.matmul(
                            psl,
                            r1lhs_sb[:, b * S + j * 128: b * S + (j + 1) * 128],
                            r1rhs_sb[:, b * H:(b + 1) * H],
                            start=True, stop=False,
                        )
                    if USE_FP8:
                        for c in range(0, HC, 2):
                            nc.tensor.matmul(
                                psl,
                                enc_t[:, c:c + 2, j * 128:(j + 1) * 128],
                                wb3[:, c:c + 2, :],
                                start=False, stop=(c + 2 == HC),
                                perf_mode=DR,
                            )
                    else:
                        for c in range(HC):
                            nc.tensor.matmul(
                                psl,
                                enc_t[:, c, j * 128:(j + 1) * 128],
                                wb3[:, c, :],
                                start=False, stop=(c == HC - 1),
                            )
                t_t = tpool.tile([128, 2 * H], BF, tag="t")
                tanh_scale = 1.0 / PSUM_SCALE if USE_FP8 else 1.0
                if split_first:
                    nc.scalar.activation(t_t[:, 0:H], ps[:, 0:H], AF.Tanh,
                                         scale=tanh_scale)
                    nc.scalar.activation(t_t[:, H:], ps[:, H:], AF.Tanh,
                                         scale=tanh_scale)
                else:
                    nc.scalar.activation(t_t[:], ps[:], AF.Tanh,
                                         scale=tanh_scale)
                scr = spool.tile([128, 2 * H], BF, tag="scr")
                nc.vector.tensor_tensor(scr[:], t_t[:], vbc_sb[:], OP.mult)
                for jj in range(2):
                    j = j0 + jj
                    scr2 = spool.tile([128, H], BF, tag="scr2")
                    nc.vector.tensor_scalar(
                        scr2[:], scr[:, jj * H:(jj + 1) * H], 1.0, None,
                        OP.mult, OP.add, accum_out=att_t[:, j:j + 1],
                    )

            # ---- masked-exp tail (tiny; the exact 1/sum normalize and the
            # cov add are a host-side elementwise epilogue) ----
            expt = small.tile([SLO, SHI], F32, tag="expt")
            nc.scalar.activation(expt[:], att_t[:], AF.Exp)
            mexp = small.tile([SLO, SHI], F32, tag="mexp")
            nc.vector.scalar_tensor_tensor(
                out=mexp[:], in0=iota_sb, scalar=lens_sb[:, b:b + 1],
                in1=expt[:], op0=OP.is_lt, op1=OP.mult,
            )
            nc.sync.dma_start(att_out[b], mexp[:])
            sum_ps = pps.tile([1, SHI], F32, tag="sum")
            nc.tensor.matmul(sum_ps[:], onesc_sb[:], mexp[:],
                             start=True, stop=True)
            ssum = small.tile([1, 1], F32, tag="ssum")
            nc.vector.reduce_sum(ssum[:], sum_ps[:], axis=mybir.AxisListType.X)
            nc.sync.dma_start(sum_out[b], ssum[:])

    nc.compile()
    return nc


def _get_nc():
    if "nc" not in _CACHE:
        _CACHE["nc"] = _build_nc()
    return _CACHE["nc"]


def _prep_in_maps(dec_input, enc_output, text_lengths, coverage_vector, W, b, v_w):
    enc = np.asarray(enc_output, dtype=np.float32)
    dec = np.asarray(dec_input, dtype=np.float32).reshape(B, E)
    cov = np.asarray(coverage_vector, dtype=np.float32)
    W = np.asarray(W, dtype=np.float32)
    b = np.asarray(b, dtype=np.float32)
    v_w = np.asarray(v_w, dtype=np.float32)
    lens_f = np.asarray(text_lengths).astype(np.float32)

    enc_dt = FP8 if USE_FP8 else BF16
    ps = PSUM_SCALE if USE_FP8 else 1.0
    es = ENC_SCALE if USE_FP8 else 1.0
    ws = W_SCALE if USE_FP8 else 1.0

    # enc^T chunk layout [B, 128, HC, S], host-cast (+pre-scale for fp8)
    encT = (enc * es if USE_FP8 else enc).reshape(B, S, HC, 128) \
        .transpose(0, 3, 2, 1)
    encT = np.ascontiguousarray(encT).astype(enc_dt).reshape(B, 128, HC * S)

    wenc = W[:H] * ws                                  # (H, H)
    wblob = np.ascontiguousarray(
        wenc.reshape(HC, 128, H).transpose(1, 0, 2).reshape(128, HC * H)
    ).astype(enc_dt)

    dec_proj = dec @ W[H:H + E] + b                    # (B, H)
    wcovsum = W[H + E:].sum(axis=0, dtype=np.float32)  # (H,)

    vbc = np.ascontiguousarray(np.broadcast_to(
        np.concatenate([v_w, v_w]).astype(BF16), (128, 2 * H)))
    onesc = np.ones((128, 1), np.float32)
    iota = (np.arange(SLO, dtype=np.float32)[:, None]
            + 128.0 * np.arange(SHI, dtype=np.float32)[None, :])

    in_maps = []
    for core in range(NCORES):
        sl = slice(core * BPC, (core + 1) * BPC)

        fblob = np.empty((SLO, SHI + BPC), np.float32)
        fblob[:, 0:SHI] = iota
        fblob[:, SHI:SHI + BPC] = lens_f[sl][None, :]

        if R1_FP8:
            r1 = np.empty((1, 2, BPC * (S + H)), np.float32)
            r1[0, 0, :BPC * S] = 1.0
            r1[0, 1, :BPC * S] = cov[sl].reshape(-1)
            r1[0, 0, BPC * S:] = (dec_proj[sl] * ps).reshape(-1)
            r1[0, 1, BPC * S:] = np.broadcast_to(wcovsum * ps, (BPC, H)).reshape(-1)
            # interleave: [lhs-pair | rhs-pair] as separate x-major blocks
            r1b = np.empty((1, 2 * BPC * (S + H)), np.float32)
            r1b[0, :2 * BPC * S] = r1[0, :, :BPC * S].reshape(-1)
            r1b[0, 2 * BPC * S:] = r1[0, :, BPC * S:].reshape(-1)
            r1 = r1b.astype(enc_dt)
        else:
            r1 = np.empty((2, BPC * (S + H)), np.float32)
            r1[0, :BPC * S] = 1.0
            r1[1, :BPC * S] = cov[sl].reshape(-1)
            r1[0, BPC * S:] = (dec_proj[sl] * ps).reshape(-1)
            r1[1, BPC * S:] = np.broadcast_to(wcovsum * ps, (BPC, H)).reshape(-1)
            r1 = r1.astype(BF16)

        in_maps.append({
            "encT": encT[sl],
            "wblob": wblob,
            "onesc": onesc,
            "fblob": fblob,
            "r1": r1,
            "vbc": vbc,
        })
    return in_maps


def kernel(dec_input, enc_output, text_lengths, coverage_vector, W, b, v_w, v_b):
    from concourse.bass_utils import run_bass_kernel_spmd

    nc = _get_nc()
    in_maps = _prep_in_maps(dec_input, enc_output, text_lengths,
                            coverage_vector, W, b, v_w)
    res = run_bass_kernel_spmd(nc, in_maps, core_ids=list(range(NCORES)))

    att = np.empty((B, S), np.float32)
    sums = np.empty((B, 1), np.float32)
    for core in range(NCORES):
        r = res.results[core]
        att[core * BPC:(core + 1) * BPC] = \
            r["att_out"].transpose(0, 2, 1).reshape(BPC, S)
        sums[core * BPC:(core + 1) * BPC] = r["sum_out"]
    att /= sums
    ncov = np.asarray(coverage_vector, dtype=np.float32) + att
    return att, ncov


# revision 34
# speedup vs baseline: 2.6056x; 1.0154x over previous
"""Fused sparse-attention kernel for Trainium2 (8 NeuronCores, data-parallel over batch).

Computation (per batch element b):
    X[s,k]  = enc[b] @ W_enc + dec_proj[b,k] + cov[b,s]*Wcovsum[k] + bias[k]
    T       = tanh(X)
    att[s]  = T @ v_w                      (+ v_b, which cancels in softmax)
    w       = softmax(att masked to s < len[b])
    new_cov = cov + w
Sharding: batch B=32 split 4-per-core across 8 cores; weights replicated.

Key layout/precision choices:
- enc is cast+transposed ON THE HOST to fp8 e4m3 [128p, HC, S] chunk layout, so
  the device does one contiguous full-rate load per batch element (no fp32
  DRAM bounce, no xbar DMA-transpose) and the main GEMM runs fp8 DoubleRow
  (K=256 per pass at 0.5 cyc/row = 2x PE throughput).
- fp8 operands are pre-scaled (enc*0.25, W_enc*16) to dodge e4m3 subnormals;
  the net *4 on psum is undone by the tanh's free scale arg. Host-emulated
  end-to-end relmax vs the fp32 reference: 6.9e-3 (gate 2e-2).
- The additive terms (dec_proj+bias, cov*Wcovsum) stay a bf16 K=2 rank-1
  matmul into the same psum group (R1_FP8 flips them to a K=2 fp8 DoubleRow
  pass at half PE cost, relmax 1.18e-2).
- dec_proj (dec @ W_s, 17 MFLOP total) and Wcovsum are host-computed.

Device pipeline, two s-tiles (=2 psum banks) per step:
  PE:  two accumulation groups into one [128, 2*512] psum pair-tile
  ACT: one tanh over the pair (amortizes the psum-access init cost) -> bf16
  DVE: one paired tensor_tensor T*v multiply (2x bf16 mode), then per s-tile
       tensor_scalar with accum_out for the free-dim reduce (4x mode). The
       obvious single scalar_tensor_tensor runs at 1x (no DVE perf-mode uop),
       so this 3-op split is ~20% faster overall.
Masked-softmax tail per batch in [s_lo=128, s_hi=16] layout: exp on ACT,
iota<len mask fused with the exp multiply on DVE, fp32 sum-matmul + 1/sum
broadcast via two tiny PE matmuls (max-subtraction skipped: |logits| <=
||v||_1 ~ 8, safely inside fp32 exp range; v_b cancels in softmax).
DMA order: batch-0 first quarter + the three first-needed consts go first so
the first matmul issues ~3us in; everything else streams behind it.
"""

import numpy as np
import ml_dtypes

B, S, H, E = 32, 2048, 512, 512
NCORES = 8
BPC = B // NCORES           # batches per core
SLO, SHI = 128, S // 128    # att tile layout: s = 128*j + p  ->  [p, j]
HC = H // 128               # h chunks
BF16 = ml_dtypes.bfloat16

USE_FP8 = True
R1_FP8 = False              # rank-1 terms as fp8 DoubleRow (cheaper PE, more err)
FP8 = ml_dtypes.float8_e4m3fn
ENC_SCALE = 0.25            # enc pre-scale (host)
W_SCALE = 16.0              # W_enc pre-scale (host)
PSUM_SCALE = ENC_SCALE * W_SCALE  # net scale on psum; undone in tanh

_CACHE = {}


def _build_nc():
    import concourse.mybir as mybir
    import concourse.tile as tile
    from concourse import bacc
    from contextlib import ExitStack

    dt = mybir.dt
    F32, BF = dt.float32, dt.bfloat16
    ENC_DT = dt.float8e4 if USE_FP8 else BF

    nc = bacc.Bacc("TRN2", target_bir_lowering=False, debug=False,
                   enable_asserts=False, num_devices=NCORES)

    # ---- DRAM I/O (per-core shapes) ----
    # encT[b, p, (j, c, si)] = enc[b, 128j+si, 128c+p]  (pre-scaled when fp8):
    # j-granular slices stay >=512B-contiguous per partition => full DMA rate
    encT = nc.dram_tensor("encT", [BPC, 128, SHI * HC * 128], ENC_DT,
                          kind="ExternalInput").ap()
    # wblob: wenc chunk c at cols [c*H, (c+1)*H): wenc[c][p, k] = W[128c+p, k]
    wblob = nc.dram_tensor("wblob", [128, HC * H], ENC_DT,
                           kind="ExternalInput").ap()
    # ones col (f32) for the softmax-sum matmul
    onesc = nc.dram_tensor("onesc", [128, 1], F32, kind="ExternalInput").ap()
    # f32 blob: [iota (SHI) | lens (BPC)]
    fblob = nc.dram_tensor("fblob", [SLO, SHI + BPC], F32,
                           kind="ExternalInput").ap()
    if R1_FP8:
        r1 = nc.dram_tensor("r1", [1, 2 * BPC * (S + H)], ENC_DT,
                            kind="ExternalInput").ap()
    else:
        # [lhs (ones,cov) BPC*S | rhs ((dec_proj+b)*PS, Wcovsum*PS) BPC*H]
        r1 = nc.dram_tensor("r1", [2, BPC * (S + H)], BF,
                            kind="ExternalInput").ap()
    vbc = nc.dram_tensor("vbc", [128, 2 * H], BF, kind="ExternalInput").ap()
    # unnormalized masked exp(att) + its sums; host divides (exact) + cov add
    att_out = nc.dram_tensor("att_out", [BPC, SLO, SHI], F32, kind="ExternalOutput").ap()
    sum_out = nc.dram_tensor("sum_out", [BPC, 1], F32, kind="ExternalOutput").ap()

    AF = mybir.ActivationFunctionType
    OP = mybir.AluOpType
    DR = mybir.MatmulPerfMode.DoubleRow

    with tile.TileContext(nc) as tc, ExitStack() as ctx:
        consts = ctx.enter_context(tc.tile_pool(name="consts", bufs=1))
        encp = ctx.enter_context(tc.tile_pool(name="encp", bufs=2))
        tpool = ctx.enter_context(tc.tile_pool(name="tpool", bufs=3))
        spool = ctx.enter_context(tc.tile_pool(name="spool", bufs=2))
        small = ctx.enter_context(tc.tile_pool(name="small", bufs=2))
        attp = ctx.enter_context(tc.tile_pool(name="attp", bufs=2))
        ppm = ctx.enter_context(tc.tile_pool(name="ppm", bufs=3, space="PSUM"))
        pps = ctx.enter_context(tc.tile_pool(name="pps", bufs=1, space="PSUM"))

        # ---- DMA order: batch-0 first quarter, then the first-needed consts,
        # then the rest of batch 0, then the remaining consts. Input DMAs ride
        # the SP (sync) queue; DMA_ENGINES serialize roughly in request order
        # so this gets the first matmul issued ~3us in. ----
        def enc_tile():
            return encp.tile([128, SHI, HC * 128], ENC_DT, tag="enc",
                             name="enc_t")

        def enc_load(e_t, b, lo, hi):
            src = encT[b].rearrange("p (j x) -> p j x", j=SHI)
            nc.sync.dma_start(e_t[:, lo:hi, :], src[:, lo:hi, :])

        # first-needed consts ride the ACT HWDGE queue (much lower fixed
        # latency than the Pool SWDGE path), smallest first, racing the SP
        # queue's j-granular enc bites, so the first matmul can go ~2us in;
        # bulk/later consts go via Pool SWDGE behind them.
        if R1_FP8:
            r1_sb = consts.tile([1, 2 * BPC * (S + H)], ENC_DT, tag="r1")
        else:
            r1_sb = consts.tile([2, BPC * (S + H)], BF, tag="r1")
        nc.scalar.dma_start(r1_sb[:], r1[:])
        e0 = enc_tile()
        enc_load(e0, 0, 0, 2)
        wb_sb = consts.tile([128, HC * H], ENC_DT, tag="wblob")
        nc.scalar.dma_start(wb_sb[:], wblob[:])
        if R1_FP8:
            r1l3 = r1_sb[:, 0:2 * BPC * S].rearrange("p (x c) -> p x c", x=2)
            r1r3 = r1_sb[:, 2 * BPC * S:].rearrange("p (x c) -> p x c", x=2)
        else:
            r1lhs_sb = r1_sb[:, 0:BPC * S]
            r1rhs_sb = r1_sb[:, BPC * S:]

        enc_load(e0, 0, 2, 6)
        enc_load(e0, 0, 6, 16)

        vbc_sb = consts.tile([128, 2 * H], BF, tag="vbc")
        nc.gpsimd.dma_start(vbc_sb[:], vbc[:])
        fb_sb = consts.tile([SLO, SHI + BPC], F32, tag="fblob")
        nc.gpsimd.dma_start(fb_sb[:], fblob[:])
        onesc_sb = consts.tile([128, 1], F32, tag="onesc")
        nc.gpsimd.dma_start(onesc_sb[:], onesc[:])

        iota_sb = fb_sb[:, 0:SHI]
        lens_sb = fb_sb[:, SHI:SHI + BPC]

        def load_batch(b):
            e_t = enc_tile()
            enc_load(e_t, b, 0, 8)
            enc_load(e_t, b, 8, 16)
            return e_t

        pre = {0: e0}
        wb3 = wb_sb[:].rearrange("p (c k) -> p c k", c=HC)

        # ---- main loop: two s-tiles (2 psum banks) per step ----
        for b in range(BPC):
            enc_t = pre.pop(b)
            if b + 1 < BPC:
                pre[b + 1] = load_batch(b + 1)

            att_t = attp.tile([SLO, SHI], F32, tag="att")
            enc4 = enc_t[:].rearrange("p j (c y) -> p j c y", c=HC)
            for j0 in range(0, SHI, 2):
                # the very first pair runs as two single-bank groups so the
                # first tanh issues one group earlier (shorter head latency)
                split_first = b == 0 and j0 == 0
                ps = ppm.tile([128, 2 * H], F32, tag="x")
                # rank-1s of both groups first: they depend only on the tiny
                # r1 blob, so at the head PE starts (and ramps) before enc lands
                for jj in range(2):
                    j = j0 + jj
                    psl = ps[:, jj * H:(jj + 1) * H]
                    if R1_FP8:
                        nc.tensor.matmul(
                            psl,
                            r1l3[:, :, b * S + j * 128: b * S + (j + 1) * 128],
                            r1r3[:, :, b * H:(b + 1) * H],
                            start=True, stop=False, perf_mode=DR,
                        )
                    else:
                        nc.tensor.matmul(
                            psl,
                            r1lhs_sb[:, b * S + j * 128: b * S + (j + 1) * 128],
                            r1rhs_sb[:, b * H:(b + 1) * H],
                            start=True, stop=False,
                        )
                for jj in range(2):
                    j = j0 + jj
                    psl = ps[:, jj * H:(jj + 1) * H]
                    if USE_FP8:
                        for c in range(0, HC, 2):
                            nc.tensor.matmul(
                                psl,
                                enc4[:, j, c:c + 2, :],
                                wb3[:, c:c + 2, :],
                                start=False, stop=(c + 2 == HC),
                                perf_mode=DR,
                            )
                    else:
                        for c in range(HC):
                            nc.tensor.matmul(
                                psl,
                                enc4[:, j, c, :],
                                wb3[:, c, :],
                                start=False, stop=(c == HC - 1),
                            )
                t_t = tpool.tile([128, 2 * H], BF, tag="t")
                tanh_scale = 1.0 / PSUM_SCALE if USE_FP8 else 1.0
                if split_first:
                    nc.scalar.activation(t_t[:, 0:H], ps[:, 0:H], AF.Tanh,
                                         scale=tanh_scale)
                    nc.scalar.activation(t_t[:, H:], ps[:, H:], AF.Tanh,
                                         scale=tanh_scale)
                else:
                    nc.scalar.activation(t_t[:], ps[:], AF.Tanh,
                                         scale=tanh_scale)
                scr = spool.tile([128, 2 * H], BF, tag="scr")
                nc.vector.tensor_tensor(scr[:], t_t[:], vbc_sb[:], OP.mult)
                for jj in range(2):
                    j = j0 + jj
                    scr2 = spool.tile([128, H], BF, tag="scr2")
                    nc.vector.tensor_scalar(
                        scr2[:], scr[:, jj * H:(jj + 1) * H], 1.0, None,
                        OP.mult, OP.add, accum_out=att_t[:, j:j + 1],
                    )

            # ---- masked-exp tail (tiny; the exact 1/sum normalize and the
            # cov add are a host-side elementwise epilogue) ----
            expt = small.tile([SLO, SHI], F32, tag="expt")
            nc.scalar.activation(expt[:], att_t[:], AF.Exp)
            mexp = small.tile([SLO, SHI], F32, tag="mexp")
            nc.vector.scalar_tensor_tensor(
                out=mexp[:], in0=iota_sb, scalar=lens_sb[:, b:b + 1],
                in1=expt[:], op0=OP.is_lt, op1=OP.mult,
            )
            nc.sync.dma_start(att_out[b], mexp[:])
            sum_ps = pps.tile([1, SHI], F32, tag="sum")
            nc.tensor.matmul(sum_ps[:], onesc_sb[:], mexp[:],
                             start=True, stop=True)
            ssum = small.tile([1, 1], F32, tag="ssum")
            nc.vector.reduce_sum(ssum[:], sum_ps[:], axis=mybir.AxisListType.X)
            nc.sync.dma_start(sum_out[b], ssum[:])

    nc.compile()
    return nc


def _get_nc():
    if "nc" not in _CACHE:
        _CACHE["nc"] = _build_nc()
    return _CACHE["nc"]


def _prep_in_maps(dec_input, enc_output, text_lengths, coverage_vector, W, b, v_w):
    enc = np.asarray(enc_output, dtype=np.float32)
    dec = np.asarray(dec_input, dtype=np.float32).reshape(B, E)
    cov = np.asarray(coverage_vector, dtype=np.float32)
    W = np.asarray(W, dtype=np.float32)
    b = np.asarray(b, dtype=np.float32)
    v_w = np.asarray(v_w, dtype=np.float32)
    lens_f = np.asarray(text_lengths).astype(np.float32)

    enc_dt = FP8 if USE_FP8 else BF16
    ps = PSUM_SCALE if USE_FP8 else 1.0
    es = ENC_SCALE if USE_FP8 else 1.0
    ws = W_SCALE if USE_FP8 else 1.0

    # enc^T layout [B, 128p, SHI, HC, 128s], host-cast (+pre-scale for fp8)
    encT = (enc * es if USE_FP8 else enc).reshape(B, SHI, 128, HC, 128) \
        .transpose(0, 4, 1, 3, 2)
    encT = np.ascontiguousarray(encT).astype(enc_dt) \
        .reshape(B, 128, SHI * HC * 128)

    wenc = W[:H] * ws                                  # (H, H)
    wblob = np.ascontiguousarray(
        wenc.reshape(HC, 128, H).transpose(1, 0, 2).reshape(128, HC * H)
    ).astype(enc_dt)

    dec_proj = dec @ W[H:H + E] + b                    # (B, H)
    wcovsum = W[H + E:].sum(axis=0, dtype=np.float32)  # (H,)

    vbc = np.ascontiguousarray(np.broadcast_to(
        np.concatenate([v_w, v_w]).astype(BF16), (128, 2 * H)))
    onesc = np.ones((128, 1), np.float32)
    iota = (np.arange(SLO, dtype=np.float32)[:, None]
            + 128.0 * np.arange(SHI, dtype=np.float32)[None, :])

    in_maps = []
    for core in range(NCORES):
        sl = slice(core * BPC, (core + 1) * BPC)

        fblob = np.empty((SLO, SHI + BPC), np.float32)
        fblob[:, 0:SHI] = iota
        fblob[:, SHI:SHI + BPC] = lens_f[sl][None, :]

        if R1_FP8:
            r1 = np.empty((1, 2, BPC * (S + H)), np.float32)
            r1[0, 0, :BPC * S] = 1.0
            r1[0, 1, :BPC * S] = cov[sl].reshape(-1)
            r1[0, 0, BPC * S:] = (dec_proj[sl] * ps).reshape(-1)
            r1[0, 1, BPC * S:] = np.broadcast_to(wcovsum * ps, (BPC, H)).reshape(-1)
            # interleave: [lhs-pair | rhs-pair] as separate x-major blocks
            r1b = np.empty((1, 2 * BPC * (S + H)), np.float32)
            r1b[0, :2 * BPC * S] = r1[0, :, :BPC * S].reshape(-1)
            r1b[0, 2 * BPC * S:] = r1[0, :, BPC * S:].reshape(-1)
            r1 = r1b.astype(enc_dt)
        else:
            r1 = np.empty((2, BPC * (S + H)), np.float32)
            r1[0, :BPC * S] = 1.0
            r1[1, :BPC * S] = cov[sl].reshape(-1)
            r1[0, BPC * S:] = (dec_proj[sl] * ps).reshape(-1)
            r1[1, BPC * S:] = np.broadcast_to(wcovsum * ps, (BPC, H)).reshape(-1)
            r1 = r1.astype(BF16)

        in_maps.append({
            "encT": encT[sl],
            "wblob": wblob,
            "onesc": onesc,
            "fblob": fblob,
            "r1": r1,
            "vbc": vbc,
        })
    return in_maps


def kernel(dec_input, enc_output, text_lengths, coverage_vector, W, b, v_w, v_b):
    from concourse.bass_utils import run_bass_kernel_spmd

    nc = _get_nc()
    in_maps = _prep_in_maps(dec_input, enc_output, text_lengths,
                            coverage_vector, W, b, v_w)
    res = run_bass_kernel_spmd(nc, in_maps, core_ids=list(range(NCORES)))

    att = np.empty((B, S), np.float32)
    sums = np.empty((B, 1), np.float32)
    for core in range(NCORES):
        r = res.results[core]
        att[core * BPC:(core + 1) * BPC] = \
            r["att_out"].transpose(0, 2, 1).reshape(BPC, S)
        sums[core * BPC:(core + 1) * BPC] = r["sum_out"]
    att /= sums
    ncov = np.asarray(coverage_vector, dtype=np.float32) + att
    return att, ncov


# revision 38
# speedup vs baseline: 2.7713x; 1.0636x over previous
"""Fused sparse-attention kernel for Trainium2 (8 NeuronCores, data-parallel over batch).

Computation (per batch element b):
    X[s,k]  = enc[b] @ W_enc + dec_proj[b,k] + cov[b,s]*Wcovsum[k] + bias[k]
    T       = tanh(X)
    att[s]  = T @ v_w                      (+ v_b, which cancels in softmax)
    w       = softmax(att masked to s < len[b])
    new_cov = cov + w
Sharding: batch B=32 split 4-per-core across 8 cores; weights replicated.

Key layout/precision choices:
- enc is cast+transposed ON THE HOST to fp8 e4m3 [128p, HC, S] chunk layout, so
  the device does one contiguous full-rate load per batch element (no fp32
  DRAM bounce, no xbar DMA-transpose) and the main GEMM runs fp8 DoubleRow
  (K=256 per pass at 0.5 cyc/row = 2x PE throughput).
- fp8 operands are pre-scaled (enc*0.25, W_enc*16) to dodge e4m3 subnormals;
  the net *4 on psum is undone by the tanh's free scale arg. Host-emulated
  end-to-end relmax vs the fp32 reference: 6.9e-3 (gate 2e-2).
- The additive terms (dec_proj+bias, cov*Wcovsum) stay a bf16 K=2 rank-1
  matmul into the same psum group (R1_FP8 flips them to a K=2 fp8 DoubleRow
  pass at half PE cost, relmax 1.18e-2).
- dec_proj (dec @ W_s, 17 MFLOP total) and Wcovsum are host-computed.

Device pipeline, two s-tiles (=2 psum banks) per step:
  PE:  two accumulation groups into one [128, 2*512] psum pair-tile
  ACT: one tanh over the pair (amortizes the psum-access init cost) -> bf16
  DVE: one paired tensor_tensor T*v multiply (2x bf16 mode), then per s-tile
       tensor_scalar with accum_out for the free-dim reduce (4x mode). The
       obvious single scalar_tensor_tensor runs at 1x (no DVE perf-mode uop),
       so this 3-op split is ~20% faster overall.
Masked-softmax tail per batch in [s_lo=128, s_hi=16] layout: exp on ACT,
iota<len mask fused with the exp multiply on DVE, fp32 sum-matmul + 1/sum
broadcast via two tiny PE matmuls (max-subtraction skipped: |logits| <=
||v||_1 ~ 8, safely inside fp32 exp range; v_b cancels in softmax).
DMA order: batch-0 first quarter + the three first-needed consts go first so
the first matmul issues ~3us in; everything else streams behind it.
"""

import numpy as np
import ml_dtypes

B, S, H, E = 32, 2048, 512, 512
NCORES = 8
BPC = B // NCORES           # batches per core
SLO, SHI = 128, S // 128    # att tile layout: s = 128*j + p  ->  [p, j]
HC = H // 128               # h chunks
BF16 = ml_dtypes.bfloat16

USE_FP8 = True
R1_FP8 = False              # rank-1 terms as fp8 DoubleRow (cheaper PE, more err)
FP8 = ml_dtypes.float8_e4m3fn
ENC_SCALE = 0.25            # enc pre-scale (host)
W_SCALE = 16.0              # W_enc pre-scale (host)
PSUM_SCALE = ENC_SCALE * W_SCALE  # net scale on psum; undone in tanh

_CACHE = {}


def _build_nc():
    import concourse.mybir as mybir
    import concourse.tile as tile
    from concourse import bacc
    from contextlib import ExitStack

    dt = mybir.dt
    F32, BF = dt.float32, dt.bfloat16
    ENC_DT = dt.float8e4 if USE_FP8 else BF

    nc = bacc.Bacc("TRN2", target_bir_lowering=False, debug=False,
                   enable_asserts=False, num_devices=NCORES)

    # ---- DRAM I/O (per-core shapes) ----
    # encT[b, p, (j, c, si)] = enc[b, 128j+si, 128c+p]  (pre-scaled when fp8):
    # j-granular slices stay >=512B-contiguous per partition => full DMA rate
    encT = nc.dram_tensor("encT", [BPC, 128, SHI * HC * 128], ENC_DT,
                          kind="ExternalInput").ap()
    # wblob: wenc chunk c at cols [c*H, (c+1)*H): wenc[c][p, k] = W[128c+p, k]
    wblob = nc.dram_tensor("wblob", [128, HC * H], ENC_DT,
                           kind="ExternalInput").ap()
    # ones col (f32) for the softmax-sum matmul
    onesc = nc.dram_tensor("onesc", [128, 1], F32, kind="ExternalInput").ap()
    # f32 blob: [iota (SHI) | lens (BPC)]
    fblob = nc.dram_tensor("fblob", [SLO, SHI + BPC], F32,
                           kind="ExternalInput").ap()
    if R1_FP8:
        r1 = nc.dram_tensor("r1", [1, 2 * BPC * (S + H)], ENC_DT,
                            kind="ExternalInput").ap()
    else:
        # [lhs (ones,cov) BPC*S | rhs ((dec_proj+b)*PS, Wcovsum*PS) BPC*H]
        r1 = nc.dram_tensor("r1", [2, BPC * (S + H)], BF,
                            kind="ExternalInput").ap()
    vbc = nc.dram_tensor("vbc", [128, 2 * H], BF, kind="ExternalInput").ap()
    # unnormalized masked exp(att) + its sums; host divides (exact) + cov add
    att_out = nc.dram_tensor("att_out", [BPC, SLO, SHI], F32, kind="ExternalOutput").ap()
    sum_out = nc.dram_tensor("sum_out", [BPC, 1], F32, kind="ExternalOutput").ap()

    AF = mybir.ActivationFunctionType
    OP = mybir.AluOpType
    DR = mybir.MatmulPerfMode.DoubleRow

    with tile.TileContext(nc) as tc, ExitStack() as ctx:
        consts = ctx.enter_context(tc.tile_pool(name="consts", bufs=1))
        encp = ctx.enter_context(tc.tile_pool(name="encp", bufs=2))
        tpool = ctx.enter_context(tc.tile_pool(name="tpool", bufs=3))
        spool = ctx.enter_context(tc.tile_pool(name="spool", bufs=2))
        small = ctx.enter_context(tc.tile_pool(name="small", bufs=2))
        attp = ctx.enter_context(tc.tile_pool(name="attp", bufs=2))
        ppm = ctx.enter_context(tc.tile_pool(name="ppm", bufs=3, space="PSUM"))
        pps = ctx.enter_context(tc.tile_pool(name="pps", bufs=1, space="PSUM"))

        # ---- DMA order: batch-0 first quarter, then the first-needed consts,
        # then the rest of batch 0, then the remaining consts. Input DMAs ride
        # the SP (sync) queue; DMA_ENGINES serialize roughly in request order
        # so this gets the first matmul issued ~3us in. ----
        def enc_tile():
            return encp.tile([128, SHI, HC * 128], ENC_DT, tag="enc",
                             name="enc_t")

        def enc_load(e_t, b, lo, hi):
            src = encT[b].rearrange("p (j x) -> p j x", j=SHI)
            nc.sync.dma_start(e_t[:, lo:hi, :], src[:, lo:hi, :])

        # first-needed consts ride the SP HWDGE queue (the ACT queue is
        # blocked by its 1.3us activation-table load at program start, and
        # the Pool SWDGE path has high fixed latency), smallest first, so the
        # first matmul can go ~2.5us in.
        if R1_FP8:
            r1_sb = consts.tile([1, 2 * BPC * (S + H)], ENC_DT, tag="r1")
        else:
            r1_sb = consts.tile([2, BPC * (S + H)], BF, tag="r1")
        nc.sync.dma_start(r1_sb[:], r1[:])
        e0 = enc_tile()
        enc_load(e0, 0, 0, 2)
        wb_sb = consts.tile([128, HC * H], ENC_DT, tag="wblob")
        nc.sync.dma_start(wb_sb[:], wblob[:])
        if R1_FP8:
            r1l3 = r1_sb[:, 0:2 * BPC * S].rearrange("p (x c) -> p x c", x=2)
            r1r3 = r1_sb[:, 2 * BPC * S:].rearrange("p (x c) -> p x c", x=2)
        else:
            r1lhs_sb = r1_sb[:, 0:BPC * S]
            r1rhs_sb = r1_sb[:, BPC * S:]

        enc_load(e0, 0, 2, 6)
        enc_load(e0, 0, 6, 16)

        vbc_sb = consts.tile([128, 2 * H], BF, tag="vbc")
        nc.gpsimd.dma_start(vbc_sb[:], vbc[:])
        fb_sb = consts.tile([SLO, SHI + BPC], F32, tag="fblob")
        nc.gpsimd.dma_start(fb_sb[:], fblob[:])
        onesc_sb = consts.tile([128, 1], F32, tag="onesc")
        nc.gpsimd.dma_start(onesc_sb[:], onesc[:])

        iota_sb = fb_sb[:, 0:SHI]
        lens_sb = fb_sb[:, SHI:SHI + BPC]

        def load_batch(b):
            e_t = enc_tile()
            enc_load(e_t, b, 0, 8)
            enc_load(e_t, b, 8, 16)
            return e_t

        pre = {0: e0}
        wb3 = wb_sb[:].rearrange("p (c k) -> p c k", c=HC)

        # ---- main loop: two s-tiles (2 psum banks) per step ----
        for b in range(BPC):
            enc_t = pre.pop(b)
            if b + 1 < BPC:
                pre[b + 1] = load_batch(b + 1)

            att_t = attp.tile([SLO, SHI], F32, tag="att")
            enc4 = enc_t[:].rearrange("p j (c y) -> p j c y", c=HC)
            for j0 in range(0, SHI, 2):
                # the very first and very last pairs run the ACT/DVE stages
                # per single s-tile: shorter pipeline-fill at the head and
                # finer pipeline-drain at the tail
                split_first = (b == 0 and j0 == 0) or \
                              (b == BPC - 1 and j0 == SHI - 2)
                ps = ppm.tile([128, 2 * H], F32, tag="x")
                # rank-1s of both groups first: they depend only on the tiny
                # r1 blob, so at the head PE starts (and ramps) before enc lands
                for jj in range(2):
                    j = j0 + jj
                    psl = ps[:, jj * H:(jj + 1) * H]
                    if R1_FP8:
                        nc.tensor.matmul(
                            psl,
                            r1l3[:, :, b * S + j * 128: b * S + (j + 1) * 128],
                            r1r3[:, :, b * H:(b + 1) * H],
                            start=True, stop=False, perf_mode=DR,
                        )
                    else:
                        nc.tensor.matmul(
                            psl,
                            r1lhs_sb[:, b * S + j * 128: b * S + (j + 1) * 128],
                            r1rhs_sb[:, b * H:(b + 1) * H],
                            start=True, stop=False,
                        )
                for jj in range(2):
                    j = j0 + jj
                    psl = ps[:, jj * H:(jj + 1) * H]
                    if USE_FP8:
                        for c in range(0, HC, 2):
                            nc.tensor.matmul(
                                psl,
                                enc4[:, j, c:c + 2, :],
                                wb3[:, c:c + 2, :],
                                start=False, stop=(c + 2 == HC),
                                perf_mode=DR,
                            )
                    else:
                        for c in range(HC):
                            nc.tensor.matmul(
                                psl,
                                enc4[:, j, c, :],
                                wb3[:, c, :],
                                start=False, stop=(c == HC - 1),
                            )
                t_t = tpool.tile([128, 2 * H], BF, tag="t")
                tanh_scale = 1.0 / PSUM_SCALE if USE_FP8 else 1.0
                scr = spool.tile([128, 2 * H], BF, tag="scr")
                if split_first:
                    for jj in range(2):
                        j = j0 + jj
                        sl = slice(jj * H, (jj + 1) * H)
                        nc.scalar.activation(t_t[:, sl], ps[:, sl], AF.Tanh,
                                             scale=tanh_scale)
                        nc.vector.tensor_tensor(scr[:, sl], t_t[:, sl],
                                                vbc_sb[:, 0:H], OP.mult)
                        scr2 = spool.tile([128, H], BF, tag="scr2")
                        nc.vector.tensor_scalar(
                            scr2[:], scr[:, sl], 1.0, None,
                            OP.mult, OP.add, accum_out=att_t[:, j:j + 1],
                        )
                else:
                    nc.scalar.activation(t_t[:], ps[:], AF.Tanh,
                                         scale=tanh_scale)
                    nc.vector.tensor_tensor(scr[:], t_t[:], vbc_sb[:], OP.mult)
                    for jj in range(2):
                        j = j0 + jj
                        scr2 = spool.tile([128, H], BF, tag="scr2")
                        nc.vector.tensor_scalar(
                            scr2[:], scr[:, jj * H:(jj + 1) * H], 1.0, None,
                            OP.mult, OP.add, accum_out=att_t[:, j:j + 1],
                        )

            # ---- masked-exp tail (tiny; the exact 1/sum normalize and the
            # cov add are a host-side elementwise epilogue). For the last
            # batch it runs in halves so the first half overlaps the final
            # s-tiles' compute instead of serializing after them. ----
            expt = small.tile([SLO, SHI], F32, tag="expt")
            mexp = small.tile([SLO, SHI], F32, tag="mexp")
            sum_ps = pps.tile([1, SHI], F32, tag="sum")
            nhalf = 2 if b == BPC - 1 else 1
            hw_ = SHI // nhalf
            for h in range(nhalf):
                sl = slice(h * hw_, (h + 1) * hw_)
                nc.scalar.activation(expt[:, sl], att_t[:, sl], AF.Exp)
                nc.vector.scalar_tensor_tensor(
                    out=mexp[:, sl], in0=iota_sb[:, sl],
                    scalar=lens_sb[:, b:b + 1],
                    in1=expt[:, sl], op0=OP.is_lt, op1=OP.mult,
                )
                nc.sync.dma_start(att_out[b][:, sl], mexp[:, sl])
                nc.tensor.matmul(sum_ps[0:1, sl], onesc_sb[:], mexp[:, sl],
                                 start=True, stop=True)
            ssum = small.tile([1, 1], F32, tag="ssum")
            nc.vector.reduce_sum(ssum[:], sum_ps[:], axis=mybir.AxisListType.X)
            nc.sync.dma_start(sum_out[b], ssum[:])

    nc.compile()
    return nc


def _get_nc():
    if "nc" not in _CACHE:
        _CACHE["nc"] = _build_nc()
    return _CACHE["nc"]


def _prep_in_maps(dec_input, enc_output, text_lengths, coverage_vector, W, b, v_w):
    enc = np.asarray(enc_output, dtype=np.float32)
    dec = np.asarray(dec_input, dtype=np.float32).reshape(B, E)
    cov = np.asarray(coverage_vector, dtype=np.float32)
    W = np.asarray(W, dtype=np.float32)
    b = np.asarray(b, dtype=np.float32)
    v_w = np.asarray(v_w, dtype=np.float32)
    lens_f = np.asarray(text_lengths).astype(np.float32)

    enc_dt = FP8 if USE_FP8 else BF16
    ps = PSUM_SCALE if USE_FP8 else 1.0
    es = ENC_SCALE if USE_FP8 else 1.0
    ws = W_SCALE if USE_FP8 else 1.0

    # enc^T layout [B, 128p, SHI, HC, 128s], host-cast (+pre-scale for fp8)
    encT = (enc * es if USE_FP8 else enc).reshape(B, SHI, 128, HC, 128) \
        .transpose(0, 4, 1, 3, 2)
    encT = np.ascontiguousarray(encT).astype(enc_dt) \
        .reshape(B, 128, SHI * HC * 128)

    wenc = W[:H] * ws                                  # (H, H)
    wblob = np.ascontiguousarray(
        wenc.reshape(HC, 128, H).transpose(1, 0, 2).reshape(128, HC * H)
    ).astype(enc_dt)

    dec_proj = dec @ W[H:H + E] + b                    # (B, H)
    wcovsum = W[H + E:].sum(axis=0, dtype=np.float32)  # (H,)

    vbc = np.ascontiguousarray(np.broadcast_to(
        np.concatenate([v_w, v_w]).astype(BF16), (128, 2 * H)))
    onesc = np.ones((128, 1), np.float32)
    iota = (np.arange(SLO, dtype=np.float32)[:, None]
            + 128.0 * np.arange(SHI, dtype=np.float32)[None, :])

    in_maps = []
    for core in range(NCORES):
        sl = slice(core * BPC, (core + 1) * BPC)

        fblob = np.empty((SLO, SHI + BPC), np.float32)
        fblob[:, 0:SHI] = iota
        fblob[:, SHI:SHI + BPC] = lens_f[sl][None, :]

        if R1_FP8:
            r1 = np.empty((1, 2, BPC * (S + H)), np.float32)
            r1[0, 0, :BPC * S] = 1.0
            r1[0, 1, :BPC * S] = cov[sl].reshape(-1)
            r1[0, 0, BPC * S:] = (dec_proj[sl] * ps).reshape(-1)
            r1[0, 1, BPC * S:] = np.broadcast_to(wcovsum * ps, (BPC, H)).reshape(-1)
            # interleave: [lhs-pair | rhs-pair] as separate x-major blocks
            r1b = np.empty((1, 2 * BPC * (S + H)), np.float32)
            r1b[0, :2 * BPC * S] = r1[0, :, :BPC * S].reshape(-1)
            r1b[0, 2 * BPC * S:] = r1[0, :, BPC * S:].reshape(-1)
            r1 = r1b.astype(enc_dt)
        else:
            r1 = np.empty((2, BPC * (S + H)), np.float32)
            r1[0, :BPC * S] = 1.0
            r1[1, :BPC * S] = cov[sl].reshape(-1)
            r1[0, BPC * S:] = (dec_proj[sl] * ps).reshape(-1)
            r1[1, BPC * S:] = np.broadcast_to(wcovsum * ps, (BPC, H)).reshape(-1)
            r1 = r1.astype(BF16)

        in_maps.append({
            "encT": encT[sl],
            "wblob": wblob,
            "onesc": onesc,
            "fblob": fblob,
            "r1": r1,
            "vbc": vbc,
        })
    return in_maps


def kernel(dec_input, enc_output, text_lengths, coverage_vector, W, b, v_w, v_b):
    from concourse.bass_utils import run_bass_kernel_spmd

    nc = _get_nc()
    in_maps = _prep_in_maps(dec_input, enc_output, text_lengths,
                            coverage_vector, W, b, v_w)
    res = run_bass_kernel_spmd(nc, in_maps, core_ids=list(range(NCORES)))

    att = np.empty((B, S), np.float32)
    sums = np.empty((B, 1), np.float32)
    for core in range(NCORES):
        r = res.results[core]
        att[core * BPC:(core + 1) * BPC] = \
            r["att_out"].transpose(0, 2, 1).reshape(BPC, S)
        sums[core * BPC:(core + 1) * BPC] = r["sum_out"]
    att /= sums
    ncov = np.asarray(coverage_vector, dtype=np.float32) + att
    return att, ncov


# revision 46
# speedup vs baseline: 2.8234x; 1.0188x over previous
"""Fused sparse-attention kernel for Trainium2 (8 NeuronCores, data-parallel over batch).

Computation (per batch element b):
    X[s,k]  = enc[b] @ W_enc + dec_proj[b,k] + cov[b,s]*Wcovsum[k] + bias[k]
    T       = tanh(X)
    att[s]  = T @ v_w                      (+ v_b, which cancels in softmax)
    w       = softmax(att masked to s < len[b])
    new_cov = cov + w
Sharding: batch B=32 split 4-per-core across 8 cores; weights replicated.

Key layout/precision choices:
- enc is cast+transposed ON THE HOST to fp8 e4m3 [128p, HC, S] chunk layout, so
  the device does one contiguous full-rate load per batch element (no fp32
  DRAM bounce, no xbar DMA-transpose) and the main GEMM runs fp8 DoubleRow
  (K=256 per pass at 0.5 cyc/row = 2x PE throughput).
- fp8 operands are pre-scaled (enc*0.25, W_enc*16) to dodge e4m3 subnormals;
  the net *4 on psum is undone by the tanh's free scale arg. Host-emulated
  end-to-end relmax vs the fp32 reference: 6.9e-3 (gate 2e-2).
- The additive terms (dec_proj+bias, cov*Wcovsum) stay a bf16 K=2 rank-1
  matmul into the same psum group (R1_FP8 flips them to a K=2 fp8 DoubleRow
  pass at half PE cost, relmax 1.18e-2).
- dec_proj (dec @ W_s, 17 MFLOP total) and Wcovsum are host-computed.

Device pipeline, two s-tiles (=2 psum banks) per step:
  PE:  two accumulation groups into one [128, 2*512] psum pair-tile
  ACT: one tanh over the pair (amortizes the psum-access init cost) -> bf16
  DVE: one paired tensor_tensor T*v multiply (2x bf16 mode), then per s-tile
       tensor_scalar with accum_out for the free-dim reduce (4x mode). The
       obvious single scalar_tensor_tensor runs at 1x (no DVE perf-mode uop),
       so this 3-op split is ~20% faster overall.
Masked-softmax tail per batch in [s_lo=128, s_hi=16] layout: exp on ACT,
iota<len mask fused with the exp multiply on DVE, fp32 sum-matmul + 1/sum
broadcast via two tiny PE matmuls (max-subtraction skipped: |logits| <=
||v||_1 ~ 8, safely inside fp32 exp range; v_b cancels in softmax).
DMA order: batch-0 first quarter + the three first-needed consts go first so
the first matmul issues ~3us in; everything else streams behind it.
"""

import numpy as np
import ml_dtypes

B, S, H, E = 32, 2048, 512, 512
NCORES = 8
BPC = B // NCORES           # batches per core
SLO, SHI = 128, S // 128    # att tile layout: s = 128*j + p  ->  [p, j]
HC = H // 128               # h chunks
BF16 = ml_dtypes.bfloat16

USE_FP8 = True
R1_FP8 = False              # rank-1 terms as fp8 DoubleRow (cheaper PE, more err)
FP8 = ml_dtypes.float8_e4m3fn
ENC_SCALE = 0.25            # enc pre-scale (host)
W_SCALE = 16.0              # W_enc pre-scale (host)
PSUM_SCALE = ENC_SCALE * W_SCALE  # net scale on psum; undone in tanh

_CACHE = {}


def _build_nc():
    import concourse.mybir as mybir
    import concourse.tile as tile
    from concourse import bacc
    from contextlib import ExitStack

    dt = mybir.dt
    F32, BF = dt.float32, dt.bfloat16
    ENC_DT = dt.float8e4 if USE_FP8 else BF

    nc = bacc.Bacc("TRN2", target_bir_lowering=False, debug=False,
                   enable_asserts=False, num_devices=NCORES)

    # ---- DRAM I/O (per-core shapes) ----
    # encT[b, p, (j, c, si)] = enc[b, 128j+si, 128c+p]  (pre-scaled when fp8):
    # j-granular slices stay >=512B-contiguous per partition => full DMA rate
    encT = nc.dram_tensor("encT", [BPC, 128, SHI * HC * 128], ENC_DT,
                          kind="ExternalInput").ap()
    # wblob: wenc chunk c at cols [c*H, (c+1)*H): wenc[c][p, k] = W[128c+p, k]
    wblob = nc.dram_tensor("wblob", [128, HC * H], ENC_DT,
                           kind="ExternalInput").ap()
    # f32 blob: [iota (SHI) | lens (BPC)]
    fblob = nc.dram_tensor("fblob", [SLO, SHI + BPC], F32,
                           kind="ExternalInput").ap()
    if R1_FP8:
        r1 = nc.dram_tensor("r1", [1, 2 * BPC * (S + H)], ENC_DT,
                            kind="ExternalInput").ap()
    else:
        # [lhs (ones,cov) BPC*S | rhs ((dec_proj+b)*PS, Wcovsum*PS) BPC*H]
        r1 = nc.dram_tensor("r1", [2, BPC * (S + H)], BF,
                            kind="ExternalInput").ap()
    vbc = nc.dram_tensor("vbc", [128, 2 * H], BF, kind="ExternalInput").ap()
    # unnormalized masked exp(att); the softmax normalize (sum + exact
    # divide) and the cov add are a host-side elementwise epilogue
    att_out = nc.dram_tensor("att_out", [BPC, SLO, SHI], F32, kind="ExternalOutput").ap()

    AF = mybir.ActivationFunctionType
    OP = mybir.AluOpType
    DR = mybir.MatmulPerfMode.DoubleRow

    with tile.TileContext(nc) as tc, ExitStack() as ctx:
        consts = ctx.enter_context(tc.tile_pool(name="consts", bufs=1))
        encp = ctx.enter_context(tc.tile_pool(name="encp", bufs=2))
        tpool = ctx.enter_context(tc.tile_pool(name="tpool", bufs=3))
        spool = ctx.enter_context(tc.tile_pool(name="spool", bufs=2))
        small = ctx.enter_context(tc.tile_pool(name="small", bufs=2))
        attp = ctx.enter_context(tc.tile_pool(name="attp", bufs=2))
        ppm = ctx.enter_context(tc.tile_pool(name="ppm", bufs=3, space="PSUM"))

        # ---- DMA order: batch-0 first quarter, then the first-needed consts,
        # then the rest of batch 0, then the remaining consts. Input DMAs ride
        # the SP (sync) queue; DMA_ENGINES serialize roughly in request order
        # so this gets the first matmul issued ~3us in. ----
        def enc_tile():
            return encp.tile([128, SHI, HC * 128], ENC_DT, tag="enc",
                             name="enc_t")

        def enc_load(e_t, b, lo, hi):
            src = encT[b].rearrange("p (j x) -> p j x", j=SHI)
            nc.sync.dma_start(e_t[:, lo:hi, :], src[:, lo:hi, :])

        # first-needed consts ride the SP HWDGE queue (the ACT queue is
        # blocked by its 1.3us activation-table load at program start, and
        # the Pool SWDGE path has high fixed latency), smallest first, so the
        # first matmul can go ~2.5us in.
        if R1_FP8:
            r1_sb = consts.tile([1, 2 * BPC * (S + H)], ENC_DT, tag="r1")
        else:
            r1_sb = consts.tile([2, BPC * (S + H)], BF, tag="r1")
        nc.sync.dma_start(r1_sb[:], r1[:])
        e0 = enc_tile()
        enc_load(e0, 0, 0, 2)
        wb_sb = consts.tile([128, HC * H], ENC_DT, tag="wblob")
        nc.sync.dma_start(wb_sb[:], wblob[:])
        if R1_FP8:
            r1l3 = r1_sb[:, 0:2 * BPC * S].rearrange("p (x c) -> p x c", x=2)
            r1r3 = r1_sb[:, 2 * BPC * S:].rearrange("p (x c) -> p x c", x=2)
        else:
            r1lhs_sb = r1_sb[:, 0:BPC * S]
            r1rhs_sb = r1_sb[:, BPC * S:]

        enc_load(e0, 0, 2, 6)
        enc_load(e0, 0, 6, 16)

        vbc_sb = consts.tile([128, 2 * H], BF, tag="vbc")
        nc.gpsimd.dma_start(vbc_sb[:], vbc[:])
        fb_sb = consts.tile([SLO, SHI + BPC], F32, tag="fblob")
        nc.gpsimd.dma_start(fb_sb[:], fblob[:])

        iota_sb = fb_sb[:, 0:SHI]
        lens_sb = fb_sb[:, SHI:SHI + BPC]

        def load_batch(b):
            e_t = enc_tile()
            enc_load(e_t, b, 0, 8)
            enc_load(e_t, b, 8, 16)
            return e_t

        pre = {0: e0}
        wb3 = wb_sb[:].rearrange("p (c k) -> p c k", c=HC)

        # ---- main loop: two s-tiles (2 psum banks) per step ----
        for b in range(BPC):
            enc_t = pre.pop(b)
            if b + 1 < BPC:
                pre[b + 1] = load_batch(b + 1)

            att_t = attp.tile([SLO, SHI], F32, tag="att")
            enc4 = enc_t[:].rearrange("p j (c y) -> p j c y", c=HC)
            for j0 in range(0, SHI, 2):
                # the very first and very last pairs run the ACT/DVE stages
                # per single s-tile: shorter pipeline-fill at the head and
                # finer pipeline-drain at the tail
                split_first = (b == 0 and j0 == 0) or \
                              (b == BPC - 1 and j0 == SHI - 2)
                ps = ppm.tile([128, 2 * H], F32, tag="x")
                # rank-1s of both groups first: they depend only on the tiny
                # r1 blob, so at the head PE starts (and ramps) before enc lands
                for jj in range(2):
                    j = j0 + jj
                    psl = ps[:, jj * H:(jj + 1) * H]
                    if R1_FP8:
                        nc.tensor.matmul(
                            psl,
                            r1l3[:, :, b * S + j * 128: b * S + (j + 1) * 128],
                            r1r3[:, :, b * H:(b + 1) * H],
                            start=True, stop=False, perf_mode=DR,
                        )
                    else:
                        nc.tensor.matmul(
                            psl,
                            r1lhs_sb[:, b * S + j * 128: b * S + (j + 1) * 128],
                            r1rhs_sb[:, b * H:(b + 1) * H],
                            start=True, stop=False,
                        )
                for jj in range(2):
                    j = j0 + jj
                    psl = ps[:, jj * H:(jj + 1) * H]
                    if USE_FP8:
                        for c in range(0, HC, 2):
                            nc.tensor.matmul(
                                psl,
                                enc4[:, j, c:c + 2, :],
                                wb3[:, c:c + 2, :],
                                start=False, stop=(c + 2 == HC),
                                perf_mode=DR,
                            )
                    else:
                        for c in range(HC):
                            nc.tensor.matmul(
                                psl,
                                enc4[:, j, c, :],
                                wb3[:, c, :],
                                start=False, stop=(c == HC - 1),
                            )
                t_t = tpool.tile([128, 2 * H], BF, tag="t")
                tanh_scale = 1.0 / PSUM_SCALE if USE_FP8 else 1.0
                scr = spool.tile([128, 2 * H], BF, tag="scr")
                if split_first:
                    for jj in range(2):
                        j = j0 + jj
                        sl = slice(jj * H, (jj + 1) * H)
                        nc.scalar.activation(t_t[:, sl], ps[:, sl], AF.Tanh,
                                             scale=tanh_scale)
                        nc.vector.tensor_tensor(scr[:, sl], t_t[:, sl],
                                                vbc_sb[:, 0:H], OP.mult)
                        scr2 = spool.tile([128, H], BF, tag="scr2")
                        nc.vector.tensor_scalar(
                            scr2[:], scr[:, sl], 1.0, None,
                            OP.mult, OP.add, accum_out=att_t[:, j:j + 1],
                        )
                else:
                    nc.scalar.activation(t_t[:], ps[:], AF.Tanh,
                                         scale=tanh_scale)
                    nc.vector.tensor_tensor(scr[:], t_t[:], vbc_sb[:], OP.mult)
                    for jj in range(2):
                        j = j0 + jj
                        scr2 = spool.tile([128, H], BF, tag="scr2")
                        nc.vector.tensor_scalar(
                            scr2[:], scr[:, jj * H:(jj + 1) * H], 1.0, None,
                            OP.mult, OP.add, accum_out=att_t[:, j:j + 1],
                        )

            # ---- masked-exp tail (tiny; the softmax sum+divide and the cov
            # add are a host-side epilogue on the returned mexp). For the
            # last batch it runs in halves so the first half overlaps the
            # final s-tiles' compute instead of serializing after them. ----
            expt = small.tile([SLO, SHI], F32, tag="expt")
            mexp = small.tile([SLO, SHI], F32, tag="mexp")
            nhalf = 2 if b == BPC - 1 else 1
            hw_ = SHI // nhalf
            for h in range(nhalf):
                sl = slice(h * hw_, (h + 1) * hw_)
                nc.scalar.activation(expt[:, sl], att_t[:, sl], AF.Exp)
                nc.vector.scalar_tensor_tensor(
                    out=mexp[:, sl], in0=iota_sb[:, sl],
                    scalar=lens_sb[:, b:b + 1],
                    in1=expt[:, sl], op0=OP.is_lt, op1=OP.mult,
                )
                nc.sync.dma_start(att_out[b][:, sl], mexp[:, sl])

    nc.compile()
    return nc


def _get_nc():
    if "nc" not in _CACHE:
        _CACHE["nc"] = _build_nc()
    return _CACHE["nc"]


def _prep_in_maps(dec_input, enc_output, text_lengths, coverage_vector, W, b, v_w):
    enc = np.asarray(enc_output, dtype=np.float32)
    dec = np.asarray(dec_input, dtype=np.float32).reshape(B, E)
    cov = np.asarray(coverage_vector, dtype=np.float32)
    W = np.asarray(W, dtype=np.float32)
    b = np.asarray(b, dtype=np.float32)
    v_w = np.asarray(v_w, dtype=np.float32)
    lens_f = np.asarray(text_lengths).astype(np.float32)

    enc_dt = FP8 if USE_FP8 else BF16
    ps = PSUM_SCALE if USE_FP8 else 1.0
    es = ENC_SCALE if USE_FP8 else 1.0
    ws = W_SCALE if USE_FP8 else 1.0

    # enc^T layout [B, 128p, SHI, HC, 128s], host-cast (+pre-scale for fp8)
    encT = (enc * es if USE_FP8 else enc).reshape(B, SHI, 128, HC, 128) \
        .transpose(0, 4, 1, 3, 2)
    encT = np.ascontiguousarray(encT).astype(enc_dt) \
        .reshape(B, 128, SHI * HC * 128)

    wenc = W[:H] * ws                                  # (H, H)
    wblob = np.ascontiguousarray(
        wenc.reshape(HC, 128, H).transpose(1, 0, 2).reshape(128, HC * H)
    ).astype(enc_dt)

    dec_proj = dec @ W[H:H + E] + b                    # (B, H)
    wcovsum = W[H + E:].sum(axis=0, dtype=np.float32)  # (H,)

    vbc = np.ascontiguousarray(np.broadcast_to(
        np.concatenate([v_w, v_w]).astype(BF16), (128, 2 * H)))
    iota = (np.arange(SLO, dtype=np.float32)[:, None]
            + 128.0 * np.arange(SHI, dtype=np.float32)[None, :])

    in_maps = []
    for core in range(NCORES):
        sl = slice(core * BPC, (core + 1) * BPC)

        fblob = np.empty((SLO, SHI + BPC), np.float32)
        fblob[:, 0:SHI] = iota
        fblob[:, SHI:SHI + BPC] = lens_f[sl][None, :]

        if R1_FP8:
            r1 = np.empty((1, 2, BPC * (S + H)), np.float32)
            r1[0, 0, :BPC * S] = 1.0
            r1[0, 1, :BPC * S] = cov[sl].reshape(-1)
            r1[0, 0, BPC * S:] = (dec_proj[sl] * ps).reshape(-1)
            r1[0, 1, BPC * S:] = np.broadcast_to(wcovsum * ps, (BPC, H)).reshape(-1)
            # interleave: [lhs-pair | rhs-pair] as separate x-major blocks
            r1b = np.empty((1, 2 * BPC * (S + H)), np.float32)
            r1b[0, :2 * BPC * S] = r1[0, :, :BPC * S].reshape(-1)
            r1b[0, 2 * BPC * S:] = r1[0, :, BPC * S:].reshape(-1)
            r1 = r1b.astype(enc_dt)
        else:
            r1 = np.empty((2, BPC * (S + H)), np.float32)
            r1[0, :BPC * S] = 1.0
            r1[1, :BPC * S] = cov[sl].reshape(-1)
            r1[0, BPC * S:] = (dec_proj[sl] * ps).reshape(-1)
            r1[1, BPC * S:] = np.broadcast_to(wcovsum * ps, (BPC, H)).reshape(-1)
            r1 = r1.astype(BF16)

        in_maps.append({
            "encT": encT[sl],
            "wblob": wblob,
            "fblob": fblob,
            "r1": r1,
            "vbc": vbc,
        })
    return in_maps


def kernel(dec_input, enc_output, text_lengths, coverage_vector, W, b, v_w, v_b):
    from concourse.bass_utils import run_bass_kernel_spmd

    nc = _get_nc()
    in_maps = _prep_in_maps(dec_input, enc_output, text_lengths,
                            coverage_vector, W, b, v_w)
    res = run_bass_kernel_spmd(nc, in_maps, core_ids=list(range(NCORES)))

    att = np.empty((B, S), np.float32)
    for core in range(NCORES):
        r = res.results[core]
        att[core * BPC:(core + 1) * BPC] = \
            r["att_out"].transpose(0, 2, 1).reshape(BPC, S)
    att /= att.sum(axis=1, keepdims=True, dtype=np.float32)
    ncov = np.asarray(coverage_vector, dtype=np.float32) + att
    return att, ncov
